# revision 1
# baseline (speedup 1.0000x reference)
"""Trainium2 Bass kernel for the latent-query attention module.

Module math (all fp32 inputs):
  Q = latent @ Wq.T; K = data @ Wk.T; V = data @ Wv.T
  S = (Q K^T)/sqrt(D); P = softmax_keys(S); out = (P V) @ Wo.T + bo

Sharding: 8 cores = 4 batches x 2 query-halves. Each core computes full
attention (all 8 heads) + output projection for its 256 queries,
recomputing K/V projections for its batch. Host gather is concatenation.

Device layout (no device transposes):
  QT [inner, q], KT [inner, keys], V [keys, inner(+ones col)],
  S^T [keys, q] in PSUM (softmax over keys = partition dim handled by a
  ones-column in V: row 64 of the [65, q] PV accumulator = denominator).
All matmuls fp32r (bf16 speed at free-dim >= 256, ~1e-4 accuracy) and
full K=128 partition-offset-0 (mixed-offset matmuls fail at runtime).
Each Q^T block packs the even head (odd rows zeroed) in cols 0:256 and
the odd head (even rows zeroed) in cols 256:512, so one N=512 S matmul
serves a head-pair off a single stationary K^T load.
Host pre-arranges every input partition-major so each DMA moves large
contiguous per-partition runs (HWDGE descriptor-gen cost ~10ns/desc).
"""

import sys

sys.path.insert(0, "/opt/trn_rl_repo")

import numpy as np

B, DS, DC = 4, 4096, 256
LS, LC = 512, 512
H, D = 8, 64
INNER, OUT_DIM = 512, 512
NCORES = 8
QPC = LS // 2          # queries per core
KB = DS // 128         # 32 key blocks of 128
SCALE = D ** -0.5

_CACHE = {}


def _emit(ctx, tc, nc, stages=5):
    from concourse import mybir
    from concourse.tile_rust import add_dep_helper

    f32 = mybir.dt.float32
    CDT = mybir.dt.float32r
    Exp = mybir.ActivationFunctionType.Exp

    # DRAM I/O - all partition-major ([128, ...] with large contiguous
    # per-partition runs; see shard() for the host-side arrangement).
    dataT = nc.dram_tensor("dataT", [128, 2 * DS], f32, kind="ExternalInput").ap()
    latentT = nc.dram_tensor("latentT", [128, 4 * QPC], f32, kind="ExternalInput").ap()
    wqT = nc.dram_tensor("wqT", [128, 4 * INNER], f32, kind="ExternalInput").ap()
    wkT = nc.dram_tensor("wkT", [128, 2 * INNER], f32, kind="ExternalInput").ap()
    wvT = nc.dram_tensor("wvT", [128, 2 * INNER], f32, kind="ExternalInput").ap()
    woT = nc.dram_tensor("woT", [128, 4 * OUT_DIM], f32, kind="ExternalInput").ap()
    bo = nc.dram_tensor("bo", [1, OUT_DIM], f32, kind="ExternalInput").ap()
    outp = nc.dram_tensor("outp", [128, 2 * OUT_DIM], f32, kind="ExternalOutput").ap()

    # ---- resident tiles ----
    res = ctx.enter_context(tc.tile_pool(name="resident", bufs=1))
    # Q^T per block: cols 0:256 even head (odd rows zero), cols 256:512
    # odd head (even rows zero) -> one N=512 S matmul per head-pair
    qt_c = res.tile([128, 4, 2 * QPC], CDT, name="qt_c")
    kt_r = res.tile([128, 4, DS], CDT, name="kt_r")    # K^T blocks
    v_r = res.tile([128, KB, H * 65], CDT, name="v_r")  # V (+ones col)
    attn_r = res.tile([128, 4, QPC], CDT, name="attn_r")
    ones_q = res.tile([1, 128], CDT, name="ones_q")
    v_view = v_r.rearrange("p k (h e) -> p k h e", e=65)

    # wof holds Wo/bo (loaded during pass 1); opened early for LIFO order
    wof = ctx.enter_context(tc.tile_pool(name="wof", bufs=1))

    # ---- attention-scope pools (alive for nearly the whole kernel) ----
    # PSUM: sps 2x2 banks + ops 2x1 + kvps 2x1 = 8 banks exactly.
    # dstage/dround/ptp double as generic staging in the front phases.
    att_ctxs = [tc.tile_pool(name="sps", bufs=2, space="PSUM"),
                tc.tile_pool(name="ops", bufs=1, space="PSUM"),
                tc.tile_pool(name="kvps", bufs=2, space="PSUM"),
                tc.tile_pool(name="ptp", bufs=3),
                tc.tile_pool(name="dstage", bufs=4),
                tc.tile_pool(name="dround", bufs=5)]
    sps, ops, kvps, ptp, dstage, dround = [c.__enter__() for c in att_ctxs]

    def close_att():
        for c in reversed(att_ctxs):
            c.__exit__(None, None, None)

    wkv_ctx = tc.tile_pool(name="wkv", bufs=1)
    wkv = wkv_ctx.__enter__()
    wk_r = wkv.tile([128, 2, INNER], CDT, name="wk_r")
    wv_r = wkv.tile([128, 2, INNER], CDT, name="wv_r")

    # ---- data chunk loading (512 keys per chunk) ----
    def load_chunk(ch):
        ds_ = dstage.tile([128, 2, 512], f32, tag="ds", name="ds_")
        for c in range(2):
            nc.sync.dma_start(
                ds_[:, c, :], dataT[:, c * DS + ch * 512:c * DS + (ch + 1) * 512])
        dr_ = dround.tile([128, 2, 512], CDT, tag="dr", name="dr_")
        nc.gpsimd.tensor_copy(dr_[:], ds_[:])
        return dr_

    def load_round(dst, src, nblk, fdim, eng):
        st = dstage.tile([128, nblk, fdim], f32, tag="ds", name="st")
        nc.sync.dma_start(st[:], src.rearrange("p (k f) -> p k f", f=fdim))
        eng(dst[:], st[:])

    drs = {}

    # ---- init constants (memset f32 staging, round via copy) ----
    if True:
        ones_sf = dstage.tile([128, KB * H], f32, tag="ds", name="ones_sf")
        nc.vector.memset(ones_sf[:], 1.0)
        nc.vector.tensor_copy(ones_q[:], ones_sf[0:1, 0:128])
        nc.vector.tensor_copy(
            v_view[:, :, :, 64:65],
            ones_sf.rearrange("p (k h o) -> p k h o", h=H, o=1))
        zeros_sf = dstage.tile([128, QPC], f32, tag="ds", name="zeros_sf")
        nc.vector.memset(zeros_sf[:], 0.0)
        for m in range(4):
            nc.vector.tensor_copy(qt_c[64:128, m, 0:QPC], zeros_sf[64:128, :])
            nc.vector.tensor_copy(qt_c[0:64, m, QPC:2 * QPC], zeros_sf[0:64, :])

        # ---- phase Q: Q^T into even/odd zero-padded copies; the two
        # [128, 512] accumulators live in the ops-pool banks ----
        qps_ = [ops.tile([128, 2 * QPC], f32, tag=f"o{pr}", name=f"qp{pr}")
                for pr in range(2)]
        first_mm = {}
        for k in range(4):
            wq_s = dstage.tile([128, INNER], f32, tag="ds", name="wq_s")
            nc.sync.dma_start(wq_s[:], wqT[:, k * INNER:(k + 1) * INNER])
            wq_rk = ptp.tile([128, INNER], CDT, tag="pt", name="wq_rk")
            nc.vector.tensor_copy(wq_rk[:], wq_s[:])
            lat_s = dstage.tile([128, QPC], f32, tag="ds", name="lat_s")
            nc.sync.dma_start(lat_s[:], latentT[:, k * QPC:(k + 1) * QPC])
            lat_rk = ptp.tile([128, QPC], CDT, tag="pt", name="lat_rk")
            nc.vector.tensor_copy(lat_rk[:], lat_s[:])
            for pr in range(2):
                for j in range(2):
                    m = 2 * pr + j
                    mm = nc.tensor.matmul(
                        qps_[pr][0:128, j * QPC:(j + 1) * QPC],
                        wq_rk[:, m * 128:(m + 1) * 128], lat_rk[:],
                        start=(k == 0 and j == 0), stop=(k == 3),
                        skip_group_check=True)
                    if k == 0:
                        if j == 0:
                            first_mm[pr] = mm
                        else:
                            add_dep_helper(mm.ins, first_mm[pr].ins, sync=False,
                                           reason="bank-clear order (Q pair)")
        for pr in range(2):
            for j in range(2):
                m = 2 * pr + j
                nc.vector.tensor_copy(qt_c[0:64, m, 0:QPC],
                                      qps_[pr][0:64, j * QPC:(j + 1) * QPC])
                nc.vector.tensor_copy(qt_c[64:128, m, QPC:2 * QPC],
                                      qps_[pr][64:128, j * QPC:(j + 1) * QPC])

    # K/V weights + first data chunks queue right after the Q-critical DMAs
    load_round(wk_r, wkT, 2, INNER, nc.scalar.copy)
    drs[0] = load_chunk(0)
    load_round(wv_r, wvT, 2, INNER, nc.scalar.copy)
    drs[1] = load_chunk(1)

    def _probe(src):
        with tc.tile_pool(name="probe", bufs=1) as pr:
            pb = pr.tile([128, OUT_DIM], f32, name="pb")
            nc.vector.tensor_copy(pb[:], src)
            nc.sync.dma_start(outp[:, 0:OUT_DIM], pb[:])

    if stages < 2:
        wkv_ctx.__exit__(None, None, None)
        close_att()
        _probe(qt_c[:, 0, :])
        return

    def kv_compute(ch, dr_, part):
        # half the K^T blocks and half the V heads per pass
        for m in ((0, 1) if part == 0 else (2, 3)):
            kp = kvps.tile([128, 512], f32, tag="kv", name="kp")
            for c in range(2):
                nc.tensor.matmul(kp[:], wk_r[:, c, m * 128:(m + 1) * 128],
                                 dr_[:, c, :], start=(c == 0), stop=(c == 1))
            nc.vector.tensor_copy(kt_r[:, m, ch * 512:(ch + 1) * 512], kp[:])
        for kb4 in range(4):
            vp = kvps.tile([128, 256], f32, tag="kv", name="vp")
            for c in range(2):
                nc.tensor.matmul(
                    vp[:], dr_[:, c, kb4 * 128:(kb4 + 1) * 128],
                    wv_r[:, c, part * 256:(part + 1) * 256],
                    start=(c == 0), stop=(c == 1))
            nc.vector.tensor_copy(
                v_view[:, ch * 4 + kb4, part * 4:(part + 1) * 4, 0:64],
                vp[:].rearrange("p (h e) -> p h e", e=64))

    def attention_kb(kb, heads, ots):
        s_t = sps.tile([128, 4 * QPC], f32, tag="st", name="s_t")
        for j2 in range(2):
            # one matmul per head-pair: shared K^T stationary, packed Q^T
            m = heads[2 * j2] // 2
            nc.tensor.matmul(
                s_t[:, j2 * 2 * QPC:(j2 + 1) * 2 * QPC],
                kt_r[:, m, kb * 128:(kb + 1) * 128], qt_c[:, m, :],
                start=True, stop=True, skip_group_check=True)
        pt = ptp.tile([128, 4 * QPC], CDT, tag="pt", name="pt")
        nc.scalar.activation(pt[:], s_t[:], Exp, scale=SCALE)
        for j, h in enumerate(heads):
            nc.tensor.matmul(
                ots[(h // 2) % 2][0:65, (h % 2) * QPC:(h % 2 + 1) * QPC],
                v_r[:, kb, h * 65:(h + 1) * 65], pt[:, j * QPC:(j + 1) * QPC],
                start=(kb == 0 and h % 2 == 0),
                stop=(kb == KB - 1 and h % 2 == 1), skip_group_check=True)

    def normalize_pass(p, ots):
        # recip(denoms) -> PE broadcast along partitions -> DVE multiply
        # transient tiles ride in the ptp slots
        rc_s = ptp.tile([1, 2, 2 * QPC], f32, tag="pt", name="rc_s")
        for mi in range(2):
            nc.vector.reciprocal(rc_s[:, mi, :], ots[mi][64:65, :])
        rc_r = ptp.tile([1, 2, 2 * QPC], CDT, tag="pt", name="rc_r")
        nc.vector.tensor_copy(rc_r[:], rc_s[:])
        for mi in range(2):
            rb_ps = kvps.tile([64, 2 * QPC], f32, tag="kv", name="rb_ps")
            nc.tensor.matmul(rb_ps[0:64, :], ones_q[0:1, 0:64], rc_r[0:1, mi, :],
                             start=True, stop=True, skip_group_check=True)
            rb_s = ptp.tile([64, 2 * QPC], f32, tag="pt", name="rb_s")
            nc.scalar.copy(rb_s[:], rb_ps[:])
            for j in range(2):
                h = 4 * p + 2 * mi + j
                nc.vector.tensor_mul(
                    attn_r[j * 64:(j + 1) * 64, h // 2, :],
                    ots[mi][0:64, j * QPC:(j + 1) * QPC],
                    rb_s[:, j * QPC:(j + 1) * QPC])

    # ---- pass 0 (heads 0-3) with fused K/V projections ----
    heads0 = [0, 1, 2, 3]
    ots0 = [ops.tile([65, 2 * QPC], f32, tag=f"o{mi}", name=f"ot0{mi}")
            for mi in range(2)]
    for ch in range(8):
        dr_ = drs.pop(ch, None) or load_chunk(ch)
        kv_compute(ch, dr_, 0)
        for i in range(4):
            attention_kb(ch * 4 + i, heads0, ots0)

    if stages < 3:
        wkv_ctx.__exit__(None, None, None)
        close_att()
        _probe(kt_r[:, 0, 0:OUT_DIM])
        return
    if stages < 4:
        with tc.tile_pool(name="probe", bufs=1) as pr:
            pb = pr.tile([128, OUT_DIM], f32, name="pb")
            nc.vector.memset(pb[:], 0.0)
            nc.vector.tensor_copy(pb[0:65, :], ots0[0][:])
            nc.sync.dma_start(outp[:, 0:OUT_DIM], pb[:])
        wkv_ctx.__exit__(None, None, None)
        close_att()
        return

    # pass-1's first K/V chunk is emitted before normalize-0 so PE stays
    # fed across the pass boundary
    dr1_first = load_chunk(0)
    kv_compute(0, dr1_first, 1)
    normalize_pass(0, ots0)

    # ---- pass 1 (heads 4-7); Wo/bo chunk loads spread between chunks ----
    wo_r = wof.tile([128, 4, OUT_DIM], CDT, name="wo_r")
    bo_r = wof.tile([1, OUT_DIM], CDT, name="bo_r")
    heads1 = [4, 5, 6, 7]
    ots1 = [ops.tile([65, 2 * QPC], f32, tag=f"o{mi}", name=f"ot1{mi}")
            for mi in range(2)]
    for ch in range(8):
        if ch > 0:
            kv_compute(ch, load_chunk(ch), 1)
        if ch < 4:
            wo_s = dstage.tile([128, OUT_DIM], f32, tag="ds", name="wo_s")
            nc.sync.dma_start(wo_s[:], woT[:, ch * OUT_DIM:(ch + 1) * OUT_DIM])
            nc.gpsimd.tensor_copy(wo_r[:, ch, :], wo_s[:])
        elif ch == 4:
            bo_s = dstage.tile([1, OUT_DIM], f32, tag="ds", name="bo_s")
            nc.sync.dma_start(bo_s[:], bo)
            nc.scalar.copy(bo_r[:], bo_s[:])
        for i in range(4):
            attention_kb(ch * 4 + i, heads1, ots1)
    normalize_pass(1, ots1)

    wkv_ctx.__exit__(None, None, None)
    close_att()

    if stages < 5:
        _probe(attn_r[:, 0:2, :])
        return

    # ---- phase F: out = attn @ Wo.T + bo ----
    with tc.tile_pool(name="fps", bufs=2, space="PSUM") as fps, \
         tc.tile_pool(name="obuf", bufs=2) as obuf:
        for qb in range(2):
            fp = fps.tile([128, OUT_DIM], f32, tag="fp", name="fp")
            for c in range(4):
                nc.tensor.matmul(
                    fp[:], attn_r[:, c, qb * 128:(qb + 1) * 128], wo_r[:, c, :],
                    start=(c == 0), stop=False)
            nc.tensor.matmul(fp[:], ones_q[0:1, :], bo_r[0:1, :],
                             start=False, stop=True)
            ob = obuf.tile([128, OUT_DIM], f32, tag="ob", name="ob")
            nc.scalar.copy(ob[:], fp[:])
            nc.sync.dma_start(outp[:, qb * OUT_DIM:(qb + 1) * OUT_DIM], ob[:])


def build(stages=5):
    key = ("nc", stages)
    if key in _CACHE:
        return _CACHE[key]
    from contextlib import ExitStack

    import concourse.tile as tile
    from concourse import bacc

    nc = bacc.Bacc("TRN2", target_bir_lowering=False, debug=False,
                   num_devices=NCORES)
    with tile.TileContext(nc) as tc:
        with ExitStack() as ctx:
            _emit(ctx, tc, nc, stages=stages)
    nc.compile()
    _CACHE[key] = nc
    return nc


def _pm(a, nblk):
    """[nblk*128, f] -> partition-major [128, nblk*f]."""
    f = a.shape[1]
    return np.ascontiguousarray(
        a.reshape(nblk, 128, f).transpose(1, 0, 2).reshape(128, nblk * f))


def shard(inputs):
    data = np.asarray(inputs["data"], dtype=np.float32)
    latent = np.asarray(inputs["latent"], dtype=np.float32)
    wq = np.asarray(inputs["Wq"], dtype=np.float32)
    wk = np.asarray(inputs["Wk"], dtype=np.float32)
    wv = np.asarray(inputs["Wv"], dtype=np.float32)
    wo = np.asarray(inputs["Wo"], dtype=np.float32)
    bo = np.asarray(inputs["bo"], dtype=np.float32).reshape(1, OUT_DIM)

    wqT = _pm(wq.T, 4)
    wkT = _pm(wk.T, 2)
    wvT = _pm(wv.T, 2)
    woT = _pm(wo.T, 4)
    dataT = [_pm(data[b].T, 2) for b in range(B)]

    in_maps = []
    for i in range(NCORES):
        b, g = i // 2, i % 2
        latT = _pm(np.ascontiguousarray(latent[b, g * QPC:(g + 1) * QPC, :].T), 4)
        in_maps.append({
            "dataT": dataT[b], "latentT": latT, "wqT": wqT, "wkT": wkT,
            "wvT": wvT, "woT": woT, "bo": bo,
        })
    return in_maps


def unshard(results):
    out = np.empty((B, LS, OUT_DIM), dtype=np.float32)
    for i in range(NCORES):
        b, g = i // 2, i % 2
        o = results[i]["outp"].reshape(128, 2, OUT_DIM).transpose(1, 0, 2)
        out[b, g * QPC:(g + 1) * QPC, :] = o.reshape(QPC, OUT_DIM)
    return out


def run(inputs, trace=False):
    from concourse import bass_utils

    nc = build()
    in_maps = shard(inputs)
    res = bass_utils.run_bass_kernel_spmd(
        nc, in_maps, core_ids=list(range(NCORES)), trace=trace)
    return unshard(res.results), res


def kernel(**inputs):
    return run(inputs)[0]



# revision 2
# speedup vs baseline: 1.0116x; 1.0116x over previous
"""Trainium2 Bass kernel for the latent-query attention module (v2).

Math (fp32 inputs):
  Q = latent @ Wq.T; K = data @ Wk.T; V = data @ Wv.T
  S = (Q K^T)/sqrt(D); P = softmax_keys(S); out = (P V) @ Wo.T + bo

Sharding: 8 cores = 4 batches x 2 head-halves (4 heads each). Each core
computes K/V projections only for its 4 heads (no duplicated work) and a
partial output  A_norm @ Wo[:, half].T (+ bo on even cores only, via a
zeros bias input on odd cores).  Host unshard = sum of the pair partials.

Per-core engine plan (cost-model balanced):
  PE   : K/V/Q proj (f32r), S via zero-packed head pairs (bf16,
         K^T stationary), PV with P^T blocks stationary and V moving
         (N=65, includes a ones-column for the softmax denominators --
         2x fewer PE cycles than the V-stationary orientation),
         transposes via identity matmul, O proj (bf16).
  ACT  : most of the softmax exp (Exp activation, PSUM->bf16) + kp copies.
  DVE  : custom 1-instruction exp correction (EXP_CORR_ANT: bitwise
         mantissa extract + linear correction of an int16 Schraudolph
         approx) + some Schraudolph P1 + normalize.
  Pool : remaining Schraudolph P1 + V copies.
Softmax denominators ride as a 65th column of V through the PV matmuls.
All DRAM inputs are declared f32r/bf16 so DMA needs no conversion pass.
"""

import sys

sys.path.insert(0, "/opt/trn_rl_repo")

import numpy as np

B, DS, DC = 4, 4096, 256
LS, LC = 512, 512
H, D = 8, 64
INNER, OUT_DIM = 512, 512
NCORES = 8
HL = 4                 # local heads per core
KB = DS // 128         # 32 key blocks
NCH = DS // 512        # 8 data chunks
SCALE = D ** -0.5

# Schraudolph (int16 / bf16-layout): i16 = trunc(A*s + B), bits = bf16 of
# ~exp(s/8); P2 corrects the (1+f) vs 2^f mantissa error.
A_P1 = 128.0 * np.log2(np.e) * SCALE
B_P1 = 127.0 * 128.0 + 0.5
MANT_MASK = 0x007F0000
U_COEF = -0.2429394
V_COEF = 0.2478516

_CACHE = {}

ROUTES = {0: "act", 1: "split_dve", 2: "split_dve", 3: "split_dve",
          4: "act", 5: "split_dve", 6: "act", 7: "split_dve"}
SPLIT_COL = 640
PIPE_N = 3


# ---------------------------------------------------------------------------
# custom DVE op: out = in0 * (1 + f*(u + v*f)),  f = mantissa(in0) in [0,1)
# ---------------------------------------------------------------------------
def _register_exp_corr():
    import concourse.dve_ops as dve_ops
    from concourse.dve_spec import AluOp, Bin, C0, C1, C2, One, Spec, lower
    from concourse.dve_uop import DveOpSpec

    if "EXP_CORR_ANT" in dve_ops._SUB_OPCODE_FOR_NAME:
        return next(op for op in dve_ops.OPS if op.name == "EXP_CORR_ANT")

    from concourse.dve_spec import Src0
    _g = Bin(AluOp.BITWISE_OR, Bin(AluOp.BITWISE_AND, Src0, C0), One)
    _x = _g - One
    _body = Src0 * (_x * (_x * C2 + C1) + One)

    def _ref(in0, in1, c0, c1, c2):
        b = in0.astype(np.float32).view(np.int32)
        c0b = np.float32(c0).view(np.int32)
        g = ((b & c0b) | np.float32(1.0).view(np.int32)).view(np.float32)
        xx = g - np.float32(1.0)
        k = np.float32(1.0) + xx * (np.float32(c1) + np.float32(c2) * xx)
        return (in0.astype(np.float32) * k).astype(np.float32)

    spec = Spec(body=_body, reference=_ref)
    shas = {}
    for ver in ("v3", "v4"):
        s = DveOpSpec(name="EXP_CORR_ANT", opcode=1, uops=lower(spec, ver=ver),
                      rd1_en=False)
        shas[ver] = s.sha(ver)
    op = dve_ops.DveOp.__new__(dve_ops.DveOp)
    object.__setattr__(op, "name", "EXP_CORR_ANT")
    object.__setattr__(op, "spec", spec)
    object.__setattr__(op, "subdim", False)
    object.__setattr__(op, "uops_sha", shas)
    object.__setattr__(op, "perf_en", {})
    dve_ops.OPS.append(op)
    dve_ops._SUB_OPCODE_FOR_NAME["EXP_CORR_ANT"] = (
        dve_ops._CUSTOM_DVE_ROW_BASE + len(dve_ops.OPS) - 1)
    dve_ops.CUSTOM_DVE_SPECS["EXP_CORR_ANT"] = spec
    return op


# ---------------------------------------------------------------------------
def _emit(ctx, tc, nc):
    from concourse import mybir
    from concourse.tile_rust import add_dep_helper

    f32 = mybir.dt.float32
    f32r = mybir.dt.float32r
    bf16 = mybir.dt.bfloat16
    i16 = mybir.dt.int16
    i32 = mybir.dt.int32
    Exp = mybir.ActivationFunctionType.Exp
    Identity = mybir.ActivationFunctionType.Identity
    MULT = mybir.AluOpType.mult
    ADD = mybir.AluOpType.add
    ISEQ = mybir.AluOpType.is_equal

    exp_corr = _register_exp_corr()
    mant_c = float(np.int32(MANT_MASK).view(np.float32))

    # ---- DRAM I/O ----
    dataT = nc.dram_tensor("dataT", [128, 2, DS], bf16, kind="ExternalInput").ap()
    latT_d = nc.dram_tensor("latentT", [128, 4, LS], bf16, kind="ExternalInput").ap()
    wqT_d = nc.dram_tensor("wqT", [128, 4, 256], bf16, kind="ExternalInput").ap()
    wkT_d = nc.dram_tensor("wkT", [128, 2, 256], bf16, kind="ExternalInput").ap()
    wvT_d = nc.dram_tensor("wvT", [128, 2, 256], bf16, kind="ExternalInput").ap()
    woT_d = nc.dram_tensor("woT", [128, 2, OUT_DIM], bf16, kind="ExternalInput").ap()
    boz_d = nc.dram_tensor("boz", [1, OUT_DIM], bf16, kind="ExternalInput").ap()
    outp = nc.dram_tensor("outp", [128, 4, OUT_DIM], bf16, kind="ExternalOutput").ap()

    # ---- resident SBUF ----
    res = ctx.enter_context(tc.tile_pool(name="res", bufs=1))
    ktT = res.tile([128, 2, DS], bf16, name="ktT")        # K^T, head pairs
    vv = res.tile([128, KB, HL * 65], bf16, name="vv")    # V + ones col
    vv_v = vv.rearrange("p k (h e) -> p k h e", e=65)
    qt = res.tile([128, 2, 2 * LS], bf16, name="qt")      # zero-packed Q^T
    ident = res.tile([128, 128], bf16, name="ident")
    ones1 = res.tile([1, 128], bf16, name="ones1")
    bo_r = res.tile([1, OUT_DIM], bf16, name="bo_r")
    bias_t = res.tile([128, 1], f32, name="bias_t")
    rden = res.tile([128, 16], f32, name="rden")
    an = res.tile([128, 4, 256], bf16, name="an")         # [q, qb, 4h*64]
    anT = res.tile([128, 2, LS], bf16, name="anT")        # [inner, c, q]

    wks = ctx.enter_context(tc.tile_pool(name="wks", bufs=1))
    latT = wks.tile([128, 4, LS], bf16, name="latT")
    wqT = wks.tile([128, 4, 256], bf16, name="wqT")
    wkT = wks.tile([128, 2, 256], bf16, name="wkT")
    wvT = wks.tile([128, 2, 256], bf16, name="wvT")
    woT = wks.tile([128, 2, OUT_DIM], bf16, name="woT")

    dstage = ctx.enter_context(tc.tile_pool(name="dstage", bufs=4))
    e16p = ctx.enter_context(tc.tile_pool(name="e16p", bufs=5))
    ptp = ctx.enter_context(tc.tile_pool(name="ptp", bufs=7))

    # ---- initial DMAs, split across 3 HWDGE queues so the front is
    # parallel: SP carries data chunks, ACT the weights, DVE the latent
    # (in lc-chunks so Q proj can start after the first chunk).
    drs = {}

    def load_chunk(ch):
        d = dstage.tile([128, 2, 512], bf16, tag="ds", name="dch")
        nc.sync.dma_start(d[:], dataT[:, :, ch * 512:(ch + 1) * 512])
        return d

    # SP queue: data chunk 0, latent chunks, then the data chunk stream.
    # ACT queue: all the weights.  Keeps the front parallel.
    nc.scalar.dma_start(wqT[:], wqT_d)
    nc.sync.dma_start(latT[:, 0:2, :], latT_d[:, 0:2, :])
    nc.sync.dma_start(latT[:, 2:4, :], latT_d[:, 2:4, :])
    nc.scalar.dma_start(wkT[:], wkT_d)
    drs[0] = load_chunk(0)
    nc.scalar.dma_start(wvT[:], wvT_d)
    drs[1] = load_chunk(1)
    drs[2] = load_chunk(2)

    def late_dmas():
        # tail-only tensors: emit mid-loop so their descriptor-gen slots
        # don't crowd the front HWDGE queue
        nc.scalar.dma_start(woT[:], woT_d)
        nc.scalar.dma_start(bo_r[:], boz_d)

    # ---- constants (ones first: the PE warmup depends only on it) ----
    nc.vector.memset(ones1[:], 1.0)
    consts = ctx.enter_context(tc.tile_pool(name="consts", bufs=1))
    iden_i = consts.tile([128, 128], i32, name="iden_i")
    nc.gpsimd.iota(iden_i[:], [[1, 128]], base=0, channel_multiplier=-1)
    nc.vector.tensor_scalar(ident[:], iden_i[:], 0, None, ISEQ)
    nc.vector.memset(bias_t[:], B_P1)
    nc.vector.memset(qt[:], 0.0)
    nc.gpsimd.memset(vv_v[:, :, :, 64:65], 1.0)

    # ---- attention-scope PSUM pools (8 banks total) ----
    # stp: 2 tiles x [128,1024] f32 = 4 banks; acc: 3 banks; kvp: 1 bank
    accp_ctx = tc.tile_pool(name="accp", bufs=1, space="PSUM")
    accp = accp_ctx.__enter__()
    att_ctxs = [tc.tile_pool(name="stpa", bufs=2, space="PSUM"),
                tc.tile_pool(name="stpb", bufs=2, space="PSUM"),
                tc.tile_pool(name="kvp", bufs=1, space="PSUM")]
    stpa, stpb, kvp = [c.__enter__() for c in att_ctxs]
    accA = accp.tile([128, 455], f32, name="accA")
    accB = accp.tile([128, 455], f32, name="accB")
    accC = accp.tile([128, 130], f32, name="accC")
    acc_tiles = [accA, accB, accC]

    def acc_region(r):
        """region r (= h*4+qb) -> (tile, col offset). 7+7+2 packing."""
        t = r // 7 if r < 14 else 2
        c = (r % 7 if r < 14 else r - 14) * 65
        return acc_tiles[t], c

    first_bank_mm = {}

    def pv_matmul(h, qb, kb, pt, j):
        r = h * 4 + qb
        t, c = acc_region(r)
        first = (kb == 0)
        is_clearing = first and (r % 7 == 0 or r == 14)
        mm = nc.tensor.matmul(
            t[:, c:c + 65],
            pt[:, qb * 128:(qb + 1) * 128],
            vv[:, kb, h * 65:h * 65 + 65],
            start=is_clearing, stop=(kb == KB - 1),
            skip_group_check=True)
        ti = t.name if hasattr(t, "name") else id(t)
        if is_clearing:
            first_bank_mm[ti] = mm
        elif first and ti in first_bank_mm:
            add_dep_helper(mm.ins, first_bank_mm[ti].ins, sync=False,
                           reason="acc bank-clear order")
        return mm

    # exp route per (kb, m): "act" = whole tile on ACT; "split_dve"/
    # "split_pool" = ACT exps cols 0:SPL while Schraudolph P1 runs on cols
    # SPL:1024 on the named engine IN PARALLEL (with only 2 st PSUM slots
    # the st-occupancy of the exp stage is the pipeline's critical chain;
    # the parallel split frees st after ~0.8us instead of 1.0-1.5us), then
    # the custom DVE P2 finishes the Schraudolph part off the chain.
    SPL = SPLIT_COL

    def route(kb, m):
        if kb >= KB - 2:
            return "act"
        i = (kb % 4) * 2 + m
        return ROUTES[i]

    def kproj(ch, dch, m):
        kp = kvp.tile([128, 512], f32, tag="kv", name="kp")
        for c in range(2):
            nc.tensor.matmul(kp[:], wkT[:, c, m * 128:(m + 1) * 128],
                             dch[:, c, :], start=(c == 0), stop=(c == 1))
        eng = nc.vector.tensor_copy if m == 0 else nc.scalar.copy
        eng(ktT[:, m, ch * 512:(ch + 1) * 512], kp[:])

    def vproj2(ch, dch, half):
        """V projection for two key blocks (one [128,512] psum tile, one
        strided copy into vv)."""
        vp = kvp.tile([128, 2, 256], f32, tag="kv", name="vp")
        for b in range(2):
            kb4 = half * 2 + b
            for c in range(2):
                nc.tensor.matmul(vp[:, b, :],
                                 dch[:, c, kb4 * 128:(kb4 + 1) * 128],
                                 wvT[:, c, :], start=(c == 0), stop=(c == 1),
                                 skip_group_check=True)
        eng = nc.scalar.copy if half == 0 else nc.vector.tensor_copy
        eng(vv_v[:, ch * 4 + half * 2:ch * 4 + half * 2 + 2, :, 0:64],
            vp[:].rearrange("p b (h e) -> p b h e", e=64))

    def s_exp(kb, m):
        """S matmuls + exp for (kb, m).  The two head-halves go to two
        INDEPENDENT psum tiles: stA (h_even) is exp'd by ACT, stB (h_odd)
        takes the Schraudolph P1 on DVE/Pool + the custom P2 -- each half
        is freed by its single reader, so neither serializes the other.
        Returns (ptA, ptB) for the PV stage."""
        rt = route(kb, m)
        sta = stpa.tile([128, LS], f32, tag="sa", name="sta")
        nc.tensor.matmul(sta[:], ktT[:, m, kb * 128:(kb + 1) * 128],
                         qt[:, m, 0:LS], start=True, stop=True,
                         skip_group_check=True)
        stb = stpb.tile([128, LS], f32, tag="sb", name="stb")
        nc.tensor.matmul(stb[:], ktT[:, m, kb * 128:(kb + 1) * 128],
                         qt[:, m, LS:2 * LS], start=True, stop=True,
                         skip_group_check=True)
        pta = ptp.tile([128, LS], bf16, tag="pa", name="pta")
        nc.scalar.activation(pta[:], sta[:], Exp, scale=SCALE)
        ptb = ptp.tile([128, LS], bf16, tag="pb", name="ptb")
        if rt == "act":
            nc.scalar.activation(ptb[:], stb[:], Exp, scale=SCALE)
        else:
            e16 = e16p.tile([128, LS], bf16, tag="e16", name="e16")
            nc.vector.tensor_scalar(e16[:].bitcast(i16), stb[:],
                                    A_P1, B_P1, MULT, ADD)
            nc.vector._custom_dve(exp_corr, out=ptb[:], in0=e16[:],
                                  s0=mant_c, s1=U_COEF, imm2=V_COEF)
        return (pta, ptb)

    def emit_pv(job):
        kb, m, (pta, ptb) = job
        for j in range(2):
            h = 2 * m + j
            for qb in range(4):
                pv_matmul(h, qb, kb, pta if j == 0 else ptb, j)

    # ---- software-pipelined main loop ----
    # K/V projection for chunk ch+1 is interleaved between the attention
    # steps of chunk ch; PV for (kb, m) is emitted one tile-slot after its
    # S/exp so the PE never waits on a just-issued exp.
    from collections import deque

    pv_q = deque()
    PIPE = PIPE_N  # pending exp tiles before PV drains

    def drain(limit):
        while len(pv_q) > limit:
            emit_pv(pv_q.popleft())

    # PE p-state warmup (depends only on ones1) while the front DMAs land
    warm_t = stpa.tile([128, LS], f32, tag="sa", name="warm")
    for w in range(24):
        nc.tensor.matmul(warm_t[:, 0:128], ones1[0:1, :], ones1[0:1, :],
                         start=(w == 0), stop=(w == 23),
                         skip_group_check=True)

    # Q projection m=0 first, then chunk-0 K proj, so the kb0 S matmul's
    # inputs (qt m0 rows + ktT chunk 0) are ready as early as possible.
    st_q = [stpa.tile([128, LS], f32, tag="sa", name="st_q0"),
            stpb.tile([128, LS], f32, tag="sb", name="st_q1")]
    for k in range(4):
        nc.tensor.matmul(st_q[0][:], wqT[:, k, 0:128], latT[:, k, :],
                         start=(k == 0), stop=(k == 3),
                         skip_group_check=True)
    kproj(0, drs[0], 0)
    nc.scalar.copy(qt[0:64, 0, 0:LS], st_q[0][0:64, :])
    nc.vector.tensor_copy(qt[64:128, 0, LS:2 * LS], st_q[0][64:128, :])
    for k in range(4):
        nc.tensor.matmul(st_q[1][:], wqT[:, k, 128:256], latT[:, k, :],
                         start=(k == 0), stop=(k == 3),
                         skip_group_check=True)
    vproj2(0, drs[0], 0)
    kproj(0, drs[0], 1)
    nc.scalar.copy(qt[0:64, 1, 0:LS], st_q[1][0:64, :])
    nc.vector.tensor_copy(qt[64:128, 1, LS:2 * LS], st_q[1][64:128, :])
    vproj2(0, drs[0], 1)

    for ch in range(NCH):
        dch = drs.pop(ch)
        if ch + 3 < NCH:
            drs[ch + 3] = load_chunk(ch + 3)
        nxt = drs.get(ch + 1)
        # 4 next-chunk projection groups, one per kb (2.1us apart, so each
        # kvp-bank copy has plenty of time before the next group needs it)
        kv_jobs = deque()
        if nxt is not None:
            kv_jobs.extend([
                lambda m=0: kproj(ch + 1, nxt, m),
                lambda: vproj2(ch + 1, nxt, 0),
                lambda m=1: kproj(ch + 1, nxt, m),
                lambda: vproj2(ch + 1, nxt, 1),
            ])
        morder = (1, 0) if ch == NCH - 1 else (0, 1)
        for i in range(4):
            kb = ch * 4 + i
            for m in morder:
                pv_q.append((kb, m, s_exp(kb, m)))
                drain(PIPE)
                if kv_jobs and m == morder[1]:
                    kv_jobs.popleft()()
        if ch == 0:
            late_dmas()
    drain(0)

    # close S/KV psum pools; acc stays alive for the normalize reads
    for c in reversed(att_ctxs):
        c.__exit__(None, None, None)

    # ---- denominators -> reciprocals ----
    for t, n0, r0 in ((accA, 7, 0), (accB, 7, 7), (accC, 2, 14)):
        tv = t.rearrange("p (n e) -> p n e", e=65)
        nc.vector.reciprocal(rden[:, r0:r0 + n0], tv[:, :, 64])

    # ---- normalize + transpose + O proj, pipelined per q-block ----
    # Within each q-block the c=0 half (heads 0-1, fed by the earlier m=0
    # accumulators) runs first; the c=1 half rides the short critical path
    # from the very last PV.  The bias matmul opens each accumulation so it
    # is never on the critical path.
    with tc.tile_pool(name="fps", bufs=2, space="PSUM") as fps, \
         tc.tile_pool(name="tps", bufs=2, space="PSUM") as tps, \
         tc.tile_pool(name="obuf", bufs=4) as obuf:
        for qb in range(4):
            fp = fps.tile([128, OUT_DIM], f32, tag="fp", name="fp")
            nc.tensor.matmul(fp[:], ones1[0:1, :], bo_r[0:1, :],
                             start=True, stop=False, skip_group_check=True)
            tp = tps.tile([128, 2, 128], f32, tag="tp", name="tp")
            for c in range(2):
                for j in range(2):
                    h = 2 * c + j
                    r = h * 4 + qb
                    t, co = acc_region(r)
                    if j == 0:
                        nc.vector.tensor_scalar(
                            an[:, qb, h * 64:(h + 1) * 64],
                            t[:, co:co + 64], rden[:, r:r + 1], None, MULT)
                    else:
                        nc.scalar.mul(an[:, qb, h * 64:(h + 1) * 64],
                                      t[:, co:co + 64], rden[:, r:r + 1])
                nc.tensor.matmul(tp[:, c, :], an[:, qb, c * 128:(c + 1) * 128],
                                 ident[:], start=True, stop=True,
                                 skip_group_check=True)
                eng = nc.vector.tensor_copy if qb % 2 else nc.scalar.copy
                eng(anT[:, c, qb * 128:(qb + 1) * 128], tp[:, c, :])
                nc.tensor.matmul(fp[:], anT[:, c, qb * 128:(qb + 1) * 128],
                                 woT[:, c, :], start=False, stop=(c == 1),
                                 skip_group_check=True)
            ob = obuf.tile([128, OUT_DIM], bf16, tag="ob", name="ob")
            if qb % 2 == 0:
                nc.scalar.copy(ob[:], fp[:])
            else:
                nc.vector.tensor_copy(ob[:], fp[:])
            nc.sync.dma_start(outp[:, qb, :], ob[:])

    accp_ctx.__exit__(None, None, None)


def build():
    if "nc" in _CACHE:
        return _CACHE["nc"]
    from contextlib import ExitStack

    import concourse.tile as tile
    from concourse import bacc

    nc = bacc.Bacc("TRN2", target_bir_lowering=False, debug=False,
                   num_devices=NCORES)
    with tile.TileContext(nc) as tc:
        with ExitStack() as ctx:
            _emit(ctx, tc, nc)
    nc.compile()
    _CACHE["nc"] = nc
    return nc


def _pm(a, nblk):
    """[nblk*128, f] -> partition-major [128, nblk, f]."""
    f = a.shape[1]
    return np.ascontiguousarray(
        a.reshape(nblk, 128, f).transpose(1, 0, 2))


def shard(inputs):
    import ml_dtypes

    data = np.asarray(inputs["data"], dtype=np.float32)
    latent = np.asarray(inputs["latent"], dtype=np.float32)
    wq = np.asarray(inputs["Wq"], dtype=np.float32)
    wk = np.asarray(inputs["Wk"], dtype=np.float32)
    wv = np.asarray(inputs["Wv"], dtype=np.float32)
    wo = np.asarray(inputs["Wo"], dtype=np.float32)
    bo = np.asarray(inputs["bo"], dtype=np.float32).reshape(1, OUT_DIM)

    bf = ml_dtypes.bfloat16
    dataTs = [_pm(data[b].T, 2).astype(bf) for b in range(B)]
    latTs = [_pm(np.ascontiguousarray(latent[b].T), 4).astype(bf)
             for b in range(B)]
    halves = []
    for g in range(2):
        hs = slice(g * 256, (g + 1) * 256)
        wqT = _pm(np.ascontiguousarray(wq[hs, :].T), 4).astype(bf)
        wkT = _pm(np.ascontiguousarray(wk[hs, :].T), 2).astype(bf)
        wvT = _pm(np.ascontiguousarray(wv[hs, :].T), 2).astype(bf)
        woT = _pm(np.ascontiguousarray(wo[:, hs].T), 2).astype(bf)
        boz = (bo if g == 0 else np.zeros_like(bo)).astype(bf)
        halves.append((wqT, wkT, wvT, woT, boz))

    in_maps = []
    for i in range(NCORES):
        b, g = i // 2, i % 2
        wqT, wkT, wvT, woT, boz = halves[g]
        in_maps.append({
            "dataT": dataTs[b], "latentT": latTs[b], "wqT": wqT,
            "wkT": wkT, "wvT": wvT, "woT": woT, "boz": boz,
        })
    return in_maps


def unshard(results):
    out = np.empty((B, LS, OUT_DIM), dtype=np.float32)
    for b in range(B):
        acc = None
        for g in range(2):
            o = results[2 * b + g]["outp"].astype(np.float32)
            o = o.transpose(1, 0, 2).reshape(LS, OUT_DIM)
            acc = o if acc is None else acc + o
        out[b] = acc
    return out


def run(inputs, trace=False):
    from concourse import bass_utils

    nc = build()
    in_maps = shard(inputs)
    res = bass_utils.run_bass_kernel_spmd(
        nc, in_maps, core_ids=list(range(NCORES)), trace=trace)
    return unshard(res.results), res


def kernel(**inputs):
    return run(inputs)[0]


# revision 4
# speedup vs baseline: 1.0146x; 1.0030x over previous
"""Trainium2 Bass kernel for the latent-query attention module (v2).

Math (fp32 inputs):
  Q = latent @ Wq.T; K = data @ Wk.T; V = data @ Wv.T
  S = (Q K^T)/sqrt(D); P = softmax_keys(S); out = (P V) @ Wo.T + bo

Sharding: 8 cores = 4 batches x 2 head-halves (4 heads each). Each core
computes K/V projections only for its 4 heads (zero duplicated work) and
a partial output  A_norm @ Wo[:, half].T (+ bo on even cores only, via a
zeros bias input on odd cores).  Host unshard = sum of the pair partials.

Per-core pipeline (streamed over 8 data chunks of 512 keys):
  PE   : K/V/Q projections; S via zero-packed head pairs (the two heads
         of a pair occupy disjoint 64-row/512-col quadrants of qt, so a
         single 128-deep matmul per half computes one head's S block);
         PV with P^T blocks STATIONARY and V moving (N=65 per matmul --
         half the PE cycles of the V-stationary orientation; the 65th V
         column of ones accumulates the softmax denominators); A^T via
         identity-matmul transposes; O projection.  All bf16.
  ACT  : exp for the h_even S half of every tile + some h_odd halves,
         kp/vv copy share, tail copies.
  DVE  : 2-pass exp for most h_odd halves: int16 Schraudolph P1
         (tensor_scalar writing the bf16 bit pattern of ~exp) + the
         custom 1-instruction P2 EXP_CORR_ANT (bitwise mantissa extract
         + linear correction, batched over tile pairs); normalize.
  Pool : SBUF-only work (memsets, iota) -- GPSIMD cannot touch PSUM.
The two S halves go to independent single-bank PSUM tiles so each is
freed by its one reader; PV emission trails S/exp by PIPE tiles.
All DRAM I/O is bf16 (except nothing -- data/latent/weights converted on
host), so no on-device dtype-conversion passes and half the DMA bytes.
"""

import sys

sys.path.insert(0, "/opt/trn_rl_repo")

import numpy as np

B, DS, DC = 4, 4096, 256
LS, LC = 512, 512
H, D = 8, 64
INNER, OUT_DIM = 512, 512
NCORES = 8
HL = 4                 # local heads per core
KB = DS // 128         # 32 key blocks
NCH = DS // 512        # 8 data chunks
SCALE = D ** -0.5

# Schraudolph (int16 / bf16-layout): i16 = trunc(A*s + B), bits = bf16 of
# ~exp(s/8); P2 corrects the (1+f) vs 2^f mantissa error.
A_P1 = 128.0 * np.log2(np.e) * SCALE
B_P1 = 127.0 * 128.0 + 0.5
MANT_MASK = 0x007F0000
U_COEF = -0.2429394
V_COEF = 0.2478516

_CACHE = {}

ROUTES = {0: "act", 1: "split_dve", 2: "split_dve", 3: "split_dve",
          4: "act", 5: "split_dve", 6: "act", 7: "split_dve"}
SPLIT_COL = 640
PIPE_N = 3


# ---------------------------------------------------------------------------
# custom DVE op: out = in0 * (1 + f*(u + v*f)),  f = mantissa(in0) in [0,1)
# ---------------------------------------------------------------------------
def _register_exp_corr():
    import concourse.dve_ops as dve_ops
    from concourse.dve_spec import AluOp, Bin, C0, C1, C2, One, Spec, lower
    from concourse.dve_uop import DveOpSpec

    if "EXP_CORR_ANT" in dve_ops._SUB_OPCODE_FOR_NAME:
        return next(op for op in dve_ops.OPS if op.name == "EXP_CORR_ANT")

    from concourse.dve_spec import Src0
    _g = Bin(AluOp.BITWISE_OR, Bin(AluOp.BITWISE_AND, Src0, C0), One)
    _x = _g - One
    _body = Src0 * (_x * (_x * C2 + C1) + One)

    def _ref(in0, in1, c0, c1, c2):
        b = in0.astype(np.float32).view(np.int32)
        c0b = np.float32(c0).view(np.int32)
        g = ((b & c0b) | np.float32(1.0).view(np.int32)).view(np.float32)
        xx = g - np.float32(1.0)
        k = np.float32(1.0) + xx * (np.float32(c1) + np.float32(c2) * xx)
        return (in0.astype(np.float32) * k).astype(np.float32)

    spec = Spec(body=_body, reference=_ref)
    shas = {}
    for ver in ("v3", "v4"):
        s = DveOpSpec(name="EXP_CORR_ANT", opcode=1, uops=lower(spec, ver=ver),
                      rd1_en=False)
        shas[ver] = s.sha(ver)
    op = dve_ops.DveOp.__new__(dve_ops.DveOp)
    object.__setattr__(op, "name", "EXP_CORR_ANT")
    object.__setattr__(op, "spec", spec)
    object.__setattr__(op, "subdim", False)
    object.__setattr__(op, "uops_sha", shas)
    object.__setattr__(op, "perf_en", {})
    dve_ops.OPS.append(op)
    dve_ops._SUB_OPCODE_FOR_NAME["EXP_CORR_ANT"] = (
        dve_ops._CUSTOM_DVE_ROW_BASE + len(dve_ops.OPS) - 1)
    dve_ops.CUSTOM_DVE_SPECS["EXP_CORR_ANT"] = spec
    return op


# ---------------------------------------------------------------------------
def _emit(ctx, tc, nc):
    from concourse import mybir
    from concourse.tile_rust import add_dep_helper

    f32 = mybir.dt.float32
    f32r = mybir.dt.float32r
    bf16 = mybir.dt.bfloat16
    i16 = mybir.dt.int16
    i32 = mybir.dt.int32
    Exp = mybir.ActivationFunctionType.Exp
    Identity = mybir.ActivationFunctionType.Identity
    MULT = mybir.AluOpType.mult
    ADD = mybir.AluOpType.add
    ISEQ = mybir.AluOpType.is_equal

    exp_corr = _register_exp_corr()
    mant_c = float(np.int32(MANT_MASK).view(np.float32))

    # ---- DRAM I/O ----
    dataT = nc.dram_tensor("dataT", [128, 2, DS], bf16, kind="ExternalInput").ap()
    latT_d = nc.dram_tensor("latentT", [128, 4, LS], bf16, kind="ExternalInput").ap()
    wqT_d = nc.dram_tensor("wqT", [128, 4, 256], bf16, kind="ExternalInput").ap()
    wkT_d = nc.dram_tensor("wkT", [128, 2, 256], bf16, kind="ExternalInput").ap()
    wvT_d = nc.dram_tensor("wvT", [128, 2, 256], bf16, kind="ExternalInput").ap()
    woT_d = nc.dram_tensor("woT", [128, 2, OUT_DIM], bf16, kind="ExternalInput").ap()
    boz_d = nc.dram_tensor("boz", [1, OUT_DIM], bf16, kind="ExternalInput").ap()
    outp = nc.dram_tensor("outp", [128, 4, OUT_DIM], bf16, kind="ExternalOutput").ap()

    # ---- resident SBUF ----
    res = ctx.enter_context(tc.tile_pool(name="res", bufs=1))
    ktT = res.tile([128, 2, DS], bf16, name="ktT")        # K^T, head pairs
    vv = res.tile([128, KB, HL * 65], bf16, name="vv")    # V + ones col
    vv_v = vv.rearrange("p k (h e) -> p k h e", e=65)
    qt = res.tile([128, 2, 2 * LS], bf16, name="qt")      # zero-packed Q^T
    ident = res.tile([128, 128], bf16, name="ident")
    ones1 = res.tile([1, 128], bf16, name="ones1")
    bo_r = res.tile([1, OUT_DIM], bf16, name="bo_r")
    bias_t = res.tile([128, 1], f32, name="bias_t")
    rden = res.tile([128, 16], f32, name="rden")
    an = res.tile([128, 4, 256], bf16, name="an")         # [q, qb, 4h*64]
    anT = res.tile([128, 2, LS], bf16, name="anT")        # [inner, c, q]

    wks = ctx.enter_context(tc.tile_pool(name="wks", bufs=1))
    latT = wks.tile([128, 4, LS], bf16, name="latT")
    wqT = wks.tile([128, 4, 256], bf16, name="wqT")
    wkT = wks.tile([128, 2, 256], bf16, name="wkT")
    wvT = wks.tile([128, 2, 256], bf16, name="wvT")
    woT = wks.tile([128, 2, OUT_DIM], bf16, name="woT")

    dstage = ctx.enter_context(tc.tile_pool(name="dstage", bufs=4))
    e16p = ctx.enter_context(tc.tile_pool(name="e16p", bufs=5))
    ptp = ctx.enter_context(tc.tile_pool(name="ptp", bufs=7))

    # ---- initial DMAs, split across 3 HWDGE queues so the front is
    # parallel: SP carries data chunks, ACT the weights, DVE the latent
    # (in lc-chunks so Q proj can start after the first chunk).
    drs = {}

    def load_chunk(ch):
        d = dstage.tile([128, 2, 512], bf16, tag="ds", name="dch")
        nc.sync.dma_start(d[:], dataT[:, :, ch * 512:(ch + 1) * 512])
        return d

    # SP queue: data chunk 0, latent chunks, then the data chunk stream.
    # ACT queue: all the weights.  Keeps the front parallel.
    nc.scalar.dma_start(wqT[:], wqT_d)
    nc.sync.dma_start(latT[:, 0:2, :], latT_d[:, 0:2, :])
    nc.sync.dma_start(latT[:, 2:4, :], latT_d[:, 2:4, :])
    nc.scalar.dma_start(wkT[:], wkT_d)
    drs[0] = load_chunk(0)
    nc.scalar.dma_start(wvT[:], wvT_d)
    drs[1] = load_chunk(1)
    drs[2] = load_chunk(2)

    def late_dmas():
        # tail-only tensors: emit mid-loop so their descriptor-gen slots
        # don't crowd the front HWDGE queue
        nc.scalar.dma_start(woT[:], woT_d)
        nc.scalar.dma_start(bo_r[:], boz_d)

    # ---- constants (ones first: the PE warmup depends only on it) ----
    nc.vector.memset(ones1[:], 1.0)
    consts = ctx.enter_context(tc.tile_pool(name="consts", bufs=1))
    iden_i = consts.tile([128, 128], i32, name="iden_i")
    nc.gpsimd.iota(iden_i[:], [[1, 128]], base=0, channel_multiplier=-1)
    nc.vector.tensor_scalar(ident[:], iden_i[:], 0, None, ISEQ)
    nc.vector.memset(bias_t[:], B_P1)
    nc.vector.memset(qt[:], 0.0)
    nc.gpsimd.memset(vv_v[:, :, :, 64:65], 1.0)

    # ---- attention-scope PSUM pools (8 banks total) ----
    # stp: 2 tiles x [128,1024] f32 = 4 banks; acc: 3 banks; kvp: 1 bank
    accp_ctx = tc.tile_pool(name="accp", bufs=1, space="PSUM")
    accp = accp_ctx.__enter__()
    att_ctxs = [tc.tile_pool(name="stpa", bufs=2, space="PSUM"),
                tc.tile_pool(name="stpb", bufs=2, space="PSUM"),
                tc.tile_pool(name="kvp", bufs=1, space="PSUM")]
    stpa, stpb, kvp = [c.__enter__() for c in att_ctxs]
    accA = accp.tile([128, 455], f32, name="accA")
    accB = accp.tile([128, 455], f32, name="accB")
    accC = accp.tile([128, 130], f32, name="accC")
    acc_tiles = [accA, accB, accC]

    def acc_region(r):
        """region r (= h*4+qb) -> (tile, col offset). 7+7+2 packing."""
        t = r // 7 if r < 14 else 2
        c = (r % 7 if r < 14 else r - 14) * 65
        return acc_tiles[t], c

    first_bank_mm = {}

    def pv_matmul(h, qb, kb, pt, j):
        r = h * 4 + qb
        t, c = acc_region(r)
        first = (kb == 0)
        is_clearing = first and (r % 7 == 0 or r == 14)
        mm = nc.tensor.matmul(
            t[:, c:c + 65],
            pt[:, qb * 128:(qb + 1) * 128],
            vv[:, kb, h * 65:h * 65 + 65],
            start=is_clearing, stop=(kb == KB - 1),
            skip_group_check=True)
        ti = t.name if hasattr(t, "name") else id(t)
        if is_clearing:
            first_bank_mm[ti] = mm
        elif first and ti in first_bank_mm:
            add_dep_helper(mm.ins, first_bank_mm[ti].ins, sync=False,
                           reason="acc bank-clear order")
        return mm

    # exp route per (kb, m): "act" = whole tile on ACT; "split_dve"/
    # "split_pool" = ACT exps cols 0:SPL while Schraudolph P1 runs on cols
    # SPL:1024 on the named engine IN PARALLEL (with only 2 st PSUM slots
    # the st-occupancy of the exp stage is the pipeline's critical chain;
    # the parallel split frees st after ~0.8us instead of 1.0-1.5us), then
    # the custom DVE P2 finishes the Schraudolph part off the chain.
    SPL = SPLIT_COL

    def route(kb, m):
        if kb >= KB - 2:
            return "act"
        i = (kb % 4) * 2 + m
        return ROUTES[i]

    def kproj(ch, dch, m):
        kp = kvp.tile([128, 512], f32, tag="kv", name="kp")
        for c in range(2):
            nc.tensor.matmul(kp[:], wkT[:, c, m * 128:(m + 1) * 128],
                             dch[:, c, :], start=(c == 0), stop=(c == 1))
        eng = nc.vector.tensor_copy if m == 0 else nc.scalar.copy
        eng(ktT[:, m, ch * 512:(ch + 1) * 512], kp[:])

    def vproj2(ch, dch, half):
        """V projection for two key blocks (one [128,512] psum tile, one
        strided copy into vv)."""
        vp = kvp.tile([128, 2, 256], f32, tag="kv", name="vp")
        for b in range(2):
            kb4 = half * 2 + b
            for c in range(2):
                nc.tensor.matmul(vp[:, b, :],
                                 dch[:, c, kb4 * 128:(kb4 + 1) * 128],
                                 wvT[:, c, :], start=(c == 0), stop=(c == 1),
                                 skip_group_check=True)
        eng = nc.scalar.copy if half == 0 else nc.vector.tensor_copy
        eng(vv_v[:, ch * 4 + half * 2:ch * 4 + half * 2 + 2, :, 0:64],
            vp[:].rearrange("p b (h e) -> p b h e", e=64))

    # pending half-done Schraudolph B-half: (e16 double tile, pt double
    # tile).  Two consecutive split tiles share one e16/pt pair so the
    # custom P2 correction runs once per PAIR ([128,1024]) on DVE.
    p2_pend = []

    def flush_p2():
        if not p2_pend:
            return
        e16d, pt2, nh = p2_pend.pop()
        nc.vector._custom_dve(exp_corr, out=pt2[:, 0:nh * LS],
                              in0=e16d[:, 0:nh * LS],
                              s0=mant_c, s1=U_COEF, imm2=V_COEF)

    def s_exp(kb, m):
        """S matmuls + exp for (kb, m).  The two head-halves go to two
        INDEPENDENT psum tiles: stA (h_even) is exp'd by ACT, stB (h_odd)
        takes the Schraudolph P1 on DVE + the custom P2 -- each half is
        freed by its single reader, so neither serializes the other.
        Returns (ptA, ptB) for the PV stage."""
        rt = route(kb, m)
        sta = stpa.tile([128, LS], f32, tag="sa", name="sta")
        nc.tensor.matmul(sta[:], ktT[:, m, kb * 128:(kb + 1) * 128],
                         qt[:, m, 0:LS], start=True, stop=True,
                         skip_group_check=True)
        stb = stpb.tile([128, LS], f32, tag="sb", name="stb")
        nc.tensor.matmul(stb[:], ktT[:, m, kb * 128:(kb + 1) * 128],
                         qt[:, m, LS:2 * LS], start=True, stop=True,
                         skip_group_check=True)
        pta = ptp.tile([128, LS], bf16, tag="pa", name="pta")
        nc.scalar.activation(pta[:], sta[:], Exp, scale=SCALE)
        if rt == "act":
            flush_p2()
            ptb = ptp.tile([128, LS], bf16, tag="pb", name="ptb")
            nc.scalar.activation(ptb[:], stb[:], Exp, scale=SCALE)
        else:
            if p2_pend:
                e16d, pt2, nh = p2_pend[0]
                nc.vector.tensor_scalar(e16d[:, LS:2 * LS].bitcast(i16),
                                        stb[:], A_P1, B_P1, MULT, ADD)
                p2_pend[0] = (e16d, pt2, 2)
                flush_p2()
                ptb = pt2[:, LS:2 * LS]
            else:
                e16d = e16p.tile([128, 2 * LS], bf16, tag="e16", name="e16")
                pt2 = ptp.tile([128, 2 * LS], bf16, tag="pb", name="pt2")
                nc.vector.tensor_scalar(e16d[:, 0:LS].bitcast(i16), stb[:],
                                        A_P1, B_P1, MULT, ADD)
                p2_pend.append((e16d, pt2, 1))
                ptb = pt2[:, 0:LS]
        return (pta, ptb)

    def emit_pv(job):
        kb, m, (pta, ptb) = job
        for j in range(2):
            h = 2 * m + j
            for qb in range(4):
                pv_matmul(h, qb, kb, pta if j == 0 else ptb, j)

    # ---- software-pipelined main loop ----
    # K/V projection for chunk ch+1 is interleaved between the attention
    # steps of chunk ch; PV for (kb, m) is emitted one tile-slot after its
    # S/exp so the PE never waits on a just-issued exp.
    from collections import deque

    pv_q = deque()
    PIPE = PIPE_N  # pending exp tiles before PV drains

    def drain(limit):
        while len(pv_q) > limit:
            emit_pv(pv_q.popleft())

    # PE p-state warmup (depends only on ones1) while the front DMAs land
    warm_t = stpa.tile([128, LS], f32, tag="sa", name="warm")
    for w in range(24):
        nc.tensor.matmul(warm_t[:, 0:128], ones1[0:1, :], ones1[0:1, :],
                         start=(w == 0), stop=(w == 23),
                         skip_group_check=True)

    # Q projection m=0 first, then chunk-0 K proj, so the kb0 S matmul's
    # inputs (qt m0 rows + ktT chunk 0) are ready as early as possible.
    st_q = [stpa.tile([128, LS], f32, tag="sa", name="st_q0"),
            stpb.tile([128, LS], f32, tag="sb", name="st_q1")]
    for k in range(4):
        nc.tensor.matmul(st_q[0][:], wqT[:, k, 0:128], latT[:, k, :],
                         start=(k == 0), stop=(k == 3),
                         skip_group_check=True)
    kproj(0, drs[0], 0)
    nc.scalar.copy(qt[0:64, 0, 0:LS], st_q[0][0:64, :])
    nc.vector.tensor_copy(qt[64:128, 0, LS:2 * LS], st_q[0][64:128, :])
    for k in range(4):
        nc.tensor.matmul(st_q[1][:], wqT[:, k, 128:256], latT[:, k, :],
                         start=(k == 0), stop=(k == 3),
                         skip_group_check=True)
    vproj2(0, drs[0], 0)
    kproj(0, drs[0], 1)
    nc.scalar.copy(qt[0:64, 1, 0:LS], st_q[1][0:64, :])
    nc.vector.tensor_copy(qt[64:128, 1, LS:2 * LS], st_q[1][64:128, :])
    vproj2(0, drs[0], 1)

    for ch in range(NCH):
        dch = drs.pop(ch)
        if ch + 3 < NCH:
            drs[ch + 3] = load_chunk(ch + 3)
        nxt = drs.get(ch + 1)
        # 4 next-chunk projection groups, one per kb (2.1us apart, so each
        # kvp-bank copy has plenty of time before the next group needs it)
        kv_jobs = deque()
        if nxt is not None:
            kv_jobs.extend([
                lambda m=0: kproj(ch + 1, nxt, m),
                lambda: vproj2(ch + 1, nxt, 0),
                lambda m=1: kproj(ch + 1, nxt, m),
                lambda: vproj2(ch + 1, nxt, 1),
            ])
        morder = (1, 0) if ch == NCH - 1 else (0, 1)
        for i in range(4):
            kb = ch * 4 + i
            for m in morder:
                pv_q.append((kb, m, s_exp(kb, m)))
                drain(PIPE)
                if kv_jobs and m == morder[1]:
                    kv_jobs.popleft()()
        if ch == 0:
            late_dmas()
    flush_p2()
    drain(0)

    # close S/KV psum pools; acc stays alive for the normalize reads
    for c in reversed(att_ctxs):
        c.__exit__(None, None, None)

    # ---- denominators -> reciprocals ----
    for t, n0, r0 in ((accA, 7, 0), (accB, 7, 7), (accC, 2, 14)):
        tv = t.rearrange("p (n e) -> p n e", e=65)
        nc.vector.reciprocal(rden[:, r0:r0 + n0], tv[:, :, 64])

    # ---- normalize + transpose + O proj, pipelined per q-block ----
    # Within each q-block the c=0 half (heads 0-1, fed by the earlier m=0
    # accumulators) runs first; the c=1 half rides the short critical path
    # from the very last PV.  The bias matmul opens each accumulation so it
    # is never on the critical path.
    with tc.tile_pool(name="fps", bufs=2, space="PSUM") as fps, \
         tc.tile_pool(name="tps", bufs=2, space="PSUM") as tps, \
         tc.tile_pool(name="obuf", bufs=4) as obuf:
        for qb in range(4):
            fp = fps.tile([128, OUT_DIM], f32, tag="fp", name="fp")
            nc.tensor.matmul(fp[:], ones1[0:1, :], bo_r[0:1, :],
                             start=True, stop=False, skip_group_check=True)
            tp = tps.tile([128, 2, 128], f32, tag="tp", name="tp")
            for c in range(2):
                for j in range(2):
                    h = 2 * c + j
                    r = h * 4 + qb
                    t, co = acc_region(r)
                    if j == 0:
                        nc.vector.tensor_scalar(
                            an[:, qb, h * 64:(h + 1) * 64],
                            t[:, co:co + 64], rden[:, r:r + 1], None, MULT)
                    else:
                        nc.scalar.mul(an[:, qb, h * 64:(h + 1) * 64],
                                      t[:, co:co + 64], rden[:, r:r + 1])
                nc.tensor.matmul(tp[:, c, :], an[:, qb, c * 128:(c + 1) * 128],
                                 ident[:], start=True, stop=True,
                                 skip_group_check=True)
                eng = nc.vector.tensor_copy if qb % 2 else nc.scalar.copy
                eng(anT[:, c, qb * 128:(qb + 1) * 128], tp[:, c, :])
                nc.tensor.matmul(fp[:], anT[:, c, qb * 128:(qb + 1) * 128],
                                 woT[:, c, :], start=False, stop=(c == 1),
                                 skip_group_check=True)
            ob = obuf.tile([128, OUT_DIM], bf16, tag="ob", name="ob")
            if qb % 2 == 0:
                nc.scalar.copy(ob[:], fp[:])
            else:
                nc.vector.tensor_copy(ob[:], fp[:])
            nc.sync.dma_start(outp[:, qb, :], ob[:])

    accp_ctx.__exit__(None, None, None)


def build():
    if "nc" in _CACHE:
        return _CACHE["nc"]
    from contextlib import ExitStack

    import concourse.tile as tile
    from concourse import bacc

    nc = bacc.Bacc("TRN2", target_bir_lowering=False, debug=False,
                   num_devices=NCORES)
    with tile.TileContext(nc) as tc:
        with ExitStack() as ctx:
            _emit(ctx, tc, nc)
    nc.compile()
    _CACHE["nc"] = nc
    return nc


def _pm(a, nblk):
    """[nblk*128, f] -> partition-major [128, nblk, f]."""
    f = a.shape[1]
    return np.ascontiguousarray(
        a.reshape(nblk, 128, f).transpose(1, 0, 2))


def shard(inputs):
    import ml_dtypes

    data = np.asarray(inputs["data"], dtype=np.float32)
    latent = np.asarray(inputs["latent"], dtype=np.float32)
    wq = np.asarray(inputs["Wq"], dtype=np.float32)
    wk = np.asarray(inputs["Wk"], dtype=np.float32)
    wv = np.asarray(inputs["Wv"], dtype=np.float32)
    wo = np.asarray(inputs["Wo"], dtype=np.float32)
    bo = np.asarray(inputs["bo"], dtype=np.float32).reshape(1, OUT_DIM)

    bf = ml_dtypes.bfloat16
    dataTs = [_pm(data[b].T, 2).astype(bf) for b in range(B)]
    latTs = [_pm(np.ascontiguousarray(latent[b].T), 4).astype(bf)
             for b in range(B)]
    halves = []
    for g in range(2):
        hs = slice(g * 256, (g + 1) * 256)
        wqT = _pm(np.ascontiguousarray(wq[hs, :].T), 4).astype(bf)
        wkT = _pm(np.ascontiguousarray(wk[hs, :].T), 2).astype(bf)
        wvT = _pm(np.ascontiguousarray(wv[hs, :].T), 2).astype(bf)
        woT = _pm(np.ascontiguousarray(wo[:, hs].T), 2).astype(bf)
        boz = (bo if g == 0 else np.zeros_like(bo)).astype(bf)
        halves.append((wqT, wkT, wvT, woT, boz))

    in_maps = []
    for i in range(NCORES):
        b, g = i // 2, i % 2
        wqT, wkT, wvT, woT, boz = halves[g]
        in_maps.append({
            "dataT": dataTs[b], "latentT": latTs[b], "wqT": wqT,
            "wkT": wkT, "wvT": wvT, "woT": woT, "boz": boz,
        })
    return in_maps


def unshard(results):
    out = np.empty((B, LS, OUT_DIM), dtype=np.float32)
    for b in range(B):
        acc = None
        for g in range(2):
            o = results[2 * b + g]["outp"].astype(np.float32)
            o = o.transpose(1, 0, 2).reshape(LS, OUT_DIM)
            acc = o if acc is None else acc + o
        out[b] = acc
    return out


def run(inputs, trace=False):
    from concourse import bass_utils

    nc = build()
    in_maps = shard(inputs)
    res = bass_utils.run_bass_kernel_spmd(
        nc, in_maps, core_ids=list(range(NCORES)), trace=trace)
    return unshard(res.results), res


def kernel(**inputs):
    return run(inputs)[0]


# revision 6
# speedup vs baseline: 1.0280x; 1.0132x over previous
"""Trainium2 Bass kernel for the latent-query attention module (v2).

Math (fp32 inputs):
  Q = latent @ Wq.T; K = data @ Wk.T; V = data @ Wv.T
  S = (Q K^T)/sqrt(D); P = softmax_keys(S); out = (P V) @ Wo.T + bo

Sharding: 8 cores = 4 batches x 2 head-halves (4 heads each). Each core
computes K/V projections only for its 4 heads (zero duplicated work) and
a partial output  A_norm @ Wo[:, half].T (+ bo on even cores only, via a
zeros bias input on odd cores).  Host unshard = sum of the pair partials.

Per-core pipeline (streamed over 8 data chunks of 512 keys):
  PE   : K/V/Q projections; S via zero-packed head pairs (the two heads
         of a pair occupy disjoint 64-row/512-col quadrants of qt, so a
         single 128-deep matmul per half computes one head's S block);
         PV with P^T blocks STATIONARY and V moving (N=65 per matmul --
         half the PE cycles of the V-stationary orientation; the 65th V
         column of ones accumulates the softmax denominators); A^T via
         identity-matmul transposes; O projection.  All bf16.
  ACT  : exp for the h_even S half of every tile + some h_odd halves,
         kp/vv copy share, tail copies.
  DVE  : 2-pass exp for most h_odd halves: int16 Schraudolph P1
         (tensor_scalar writing the bf16 bit pattern of ~exp) + the
         custom 1-instruction P2 EXP_CORR_ANT (bitwise mantissa extract
         + linear correction, batched over tile pairs); normalize.
  Pool : SBUF-only work (memsets, iota) -- GPSIMD cannot touch PSUM.
The two S halves go to independent single-bank PSUM tiles so each is
freed by its one reader; PV emission trails S/exp by PIPE tiles.
All DRAM I/O is bf16 (except nothing -- data/latent/weights converted on
host), so no on-device dtype-conversion passes and half the DMA bytes.
"""

import sys

sys.path.insert(0, "/opt/trn_rl_repo")

import numpy as np

B, DS, DC = 4, 4096, 256
LS, LC = 512, 512
H, D = 8, 64
INNER, OUT_DIM = 512, 512
NCORES = 8
HL = 4                 # local heads per core
KB = DS // 128         # 32 key blocks
NCH = DS // 512        # 8 data chunks
SCALE = D ** -0.5

# Schraudolph (int16 / bf16-layout): i16 = trunc(A*s + B), bits = bf16 of
# ~exp(s/8); P2 corrects the (1+f) vs 2^f mantissa error.
A_P1 = 128.0 * np.log2(np.e) * SCALE
B_P1 = 127.0 * 128.0 + 0.5
MANT_MASK = 0x007F0000
U_COEF = -0.2429394
V_COEF = 0.2478516

_CACHE = {}

ROUTES = {0: "act", 1: "split_dve", 2: "split_dve", 3: "split_dve",
          4: "act", 5: "split_dve", 6: "act", 7: "split_dve"}
SPLIT_COL = 640
PIPE_N = 3


# ---------------------------------------------------------------------------
# custom DVE op: out = in0 * (1 + f*(u + v*f)),  f = mantissa(in0) in [0,1)
# ---------------------------------------------------------------------------
def _register_exp_corr():
    import concourse.dve_ops as dve_ops
    from concourse.dve_spec import AluOp, Bin, C0, C1, C2, One, Spec, lower
    from concourse.dve_uop import DveOpSpec

    if "EXP_CORR_ANT" in dve_ops._SUB_OPCODE_FOR_NAME:
        return next(op for op in dve_ops.OPS if op.name == "EXP_CORR_ANT")

    from concourse.dve_spec import Src0
    _g = Bin(AluOp.BITWISE_OR, Bin(AluOp.BITWISE_AND, Src0, C0), One)
    _x = _g - One
    _body = Src0 * (_x * (_x * C2 + C1) + One)

    def _ref(in0, in1, c0, c1, c2):
        b = in0.astype(np.float32).view(np.int32)
        c0b = np.float32(c0).view(np.int32)
        g = ((b & c0b) | np.float32(1.0).view(np.int32)).view(np.float32)
        xx = g - np.float32(1.0)
        k = np.float32(1.0) + xx * (np.float32(c1) + np.float32(c2) * xx)
        return (in0.astype(np.float32) * k).astype(np.float32)

    spec = Spec(body=_body, reference=_ref)
    shas = {}
    for ver in ("v3", "v4"):
        s = DveOpSpec(name="EXP_CORR_ANT", opcode=1, uops=lower(spec, ver=ver),
                      rd1_en=False)
        shas[ver] = s.sha(ver)
    op = dve_ops.DveOp.__new__(dve_ops.DveOp)
    object.__setattr__(op, "name", "EXP_CORR_ANT")
    object.__setattr__(op, "spec", spec)
    object.__setattr__(op, "subdim", False)
    object.__setattr__(op, "uops_sha", shas)
    object.__setattr__(op, "perf_en", {})
    dve_ops.OPS.append(op)
    dve_ops._SUB_OPCODE_FOR_NAME["EXP_CORR_ANT"] = (
        dve_ops._CUSTOM_DVE_ROW_BASE + len(dve_ops.OPS) - 1)
    dve_ops.CUSTOM_DVE_SPECS["EXP_CORR_ANT"] = spec
    return op


# ---------------------------------------------------------------------------
def _emit(ctx, tc, nc):
    from concourse import mybir
    from concourse.tile_rust import add_dep_helper

    f32 = mybir.dt.float32
    f32r = mybir.dt.float32r
    bf16 = mybir.dt.bfloat16
    i16 = mybir.dt.int16
    i32 = mybir.dt.int32
    Exp = mybir.ActivationFunctionType.Exp
    Identity = mybir.ActivationFunctionType.Identity
    MULT = mybir.AluOpType.mult
    ADD = mybir.AluOpType.add
    ISEQ = mybir.AluOpType.is_equal

    exp_corr = _register_exp_corr()
    mant_c = float(np.int32(MANT_MASK).view(np.float32))

    # ---- DRAM I/O ----
    dataT = nc.dram_tensor("dataT", [128, 2, DS], bf16, kind="ExternalInput").ap()
    latT_d = nc.dram_tensor("latentT", [128, 4, LS], bf16, kind="ExternalInput").ap()
    wqT_d = nc.dram_tensor("wqT", [128, 4, 256], bf16, kind="ExternalInput").ap()
    wkT_d = nc.dram_tensor("wkT", [128, 2, 256], bf16, kind="ExternalInput").ap()
    wvT_d = nc.dram_tensor("wvT", [128, 2, 256], bf16, kind="ExternalInput").ap()
    woT_d = nc.dram_tensor("woT", [128, 2, OUT_DIM], bf16, kind="ExternalInput").ap()
    boz_d = nc.dram_tensor("boz", [1, OUT_DIM], bf16, kind="ExternalInput").ap()
    outp = nc.dram_tensor("outp", [128, 4, OUT_DIM], bf16, kind="ExternalOutput").ap()

    # ---- resident SBUF ----
    res = ctx.enter_context(tc.tile_pool(name="res", bufs=1))
    ktT = res.tile([128, 2, DS], bf16, name="ktT")        # K^T, head pairs
    vv = res.tile([128, KB, HL * 65], bf16, name="vv")    # V + ones col
    vv_v = vv.rearrange("p k (h e) -> p k h e", e=65)
    qt = res.tile([128, 2, 2 * LS], bf16, name="qt")      # zero-packed Q^T
    ident = res.tile([128, 128], bf16, name="ident")
    ones1 = res.tile([1, 128], bf16, name="ones1")
    bo_r = res.tile([1, OUT_DIM], bf16, name="bo_r")
    bias_t = res.tile([128, 1], f32, name="bias_t")
    rden = res.tile([128, 16], f32, name="rden")
    an = res.tile([128, 4, 256], bf16, name="an")         # [q, qb, 4h*64]
    anT = res.tile([128, 2, LS], bf16, name="anT")        # [inner, c, q]

    wks = ctx.enter_context(tc.tile_pool(name="wks", bufs=1))
    latT = wks.tile([128, 4, LS], bf16, name="latT")
    wqT = wks.tile([128, 4, 256], bf16, name="wqT")
    wkT = wks.tile([128, 2, 256], bf16, name="wkT")
    wvT = wks.tile([128, 2, 256], bf16, name="wvT")
    woT = wks.tile([128, 2, OUT_DIM], bf16, name="woT")

    dstage = ctx.enter_context(tc.tile_pool(name="dstage", bufs=4))
    e16p = ctx.enter_context(tc.tile_pool(name="e16p", bufs=5))
    ptp = ctx.enter_context(tc.tile_pool(name="ptp", bufs=7))

    # ---- initial DMAs, split across 3 HWDGE queues so the front is
    # parallel: SP carries data chunks, ACT the weights, DVE the latent
    # (in lc-chunks so Q proj can start after the first chunk).
    drs = {}

    def load_chunk(ch):
        d = dstage.tile([128, 2, 512], bf16, tag="ds", name="dch")
        nc.sync.dma_start(d[:], dataT[:, :, ch * 512:(ch + 1) * 512])
        return d

    # SP queue: data chunk 0, latent chunks, then the data chunk stream.
    # ACT queue: all the weights.  Keeps the front parallel.
    nc.scalar.dma_start(wqT[:], wqT_d)
    nc.sync.dma_start(latT[:, 0:2, :], latT_d[:, 0:2, :])
    nc.sync.dma_start(latT[:, 2:4, :], latT_d[:, 2:4, :])
    nc.scalar.dma_start(wkT[:], wkT_d)
    drs[0] = load_chunk(0)
    nc.scalar.dma_start(wvT[:], wvT_d)
    drs[1] = load_chunk(1)
    drs[2] = load_chunk(2)

    def late_dmas():
        # tail-only tensors: emit mid-loop so their descriptor-gen slots
        # don't crowd the front HWDGE queue
        nc.scalar.dma_start(woT[:], woT_d)
        nc.scalar.dma_start(bo_r[:], boz_d)

    # ---- constants (ones first: the PE warmup depends only on it) ----
    nc.vector.memset(ones1[:], 1.0)
    consts = ctx.enter_context(tc.tile_pool(name="consts", bufs=1))
    iden_i = consts.tile([128, 128], i32, name="iden_i")
    nc.gpsimd.iota(iden_i[:], [[1, 128]], base=0, channel_multiplier=-1)
    nc.vector.tensor_scalar(ident[:], iden_i[:], 0, None, ISEQ)
    nc.vector.memset(bias_t[:], B_P1)
    nc.vector.memset(qt[:], 0.0)
    nc.gpsimd.memset(vv_v[:, :, :, 64:65], 1.0)

    # ---- attention-scope PSUM pools (8 banks total) ----
    # stp: 2 tiles x [128,1024] f32 = 4 banks; acc: 3 banks; kvp: 1 bank
    accp_ctx = tc.tile_pool(name="accp", bufs=1, space="PSUM")
    accp = accp_ctx.__enter__()
    att_ctxs = [tc.tile_pool(name="stpa", bufs=2, space="PSUM"),
                tc.tile_pool(name="stpb", bufs=2, space="PSUM"),
                tc.tile_pool(name="kvp", bufs=1, space="PSUM")]
    stpa, stpb, kvp = [c.__enter__() for c in att_ctxs]
    accA = accp.tile([128, 455], f32, name="accA")
    accB = accp.tile([128, 455], f32, name="accB")
    accC = accp.tile([128, 130], f32, name="accC")
    acc_tiles = [accA, accB, accC]

    def acc_region(r):
        """region r (= h*4+qb) -> (tile, col offset). 7+7+2 packing."""
        t = r // 7 if r < 14 else 2
        c = (r % 7 if r < 14 else r - 14) * 65
        return acc_tiles[t], c

    first_bank_mm = {}

    def pv_matmul(h, qb, kb, pt, j):
        r = h * 4 + qb
        t, c = acc_region(r)
        first = (kb == 0)
        is_clearing = first and (r % 7 == 0 or r == 14)
        mm = nc.tensor.matmul(
            t[:, c:c + 65],
            pt[:, qb * 128:(qb + 1) * 128],
            vv[:, kb, h * 65:h * 65 + 65],
            start=is_clearing, stop=(kb == KB - 1),
            skip_group_check=True)
        ti = t.name if hasattr(t, "name") else id(t)
        if is_clearing:
            first_bank_mm[ti] = mm
        elif first and ti in first_bank_mm:
            add_dep_helper(mm.ins, first_bank_mm[ti].ins, sync=False,
                           reason="acc bank-clear order")
        return mm

    # exp route per (kb, m): "act" = whole tile on ACT; "split_dve"/
    # "split_pool" = ACT exps cols 0:SPL while Schraudolph P1 runs on cols
    # SPL:1024 on the named engine IN PARALLEL (with only 2 st PSUM slots
    # the st-occupancy of the exp stage is the pipeline's critical chain;
    # the parallel split frees st after ~0.8us instead of 1.0-1.5us), then
    # the custom DVE P2 finishes the Schraudolph part off the chain.
    SPL = SPLIT_COL

    def route(kb, m):
        if kb >= KB - 2:
            return "act"
        i = (kb % 4) * 2 + m
        return ROUTES[i]

    def kproj(ch, dch, m):
        kp = kvp.tile([128, 512], f32, tag="kv", name="kp")
        for c in range(2):
            nc.tensor.matmul(kp[:], wkT[:, c, m * 128:(m + 1) * 128],
                             dch[:, c, :], start=(c == 0), stop=(c == 1))
        eng = nc.vector.tensor_copy if m == 0 else nc.scalar.copy
        eng(ktT[:, m, ch * 512:(ch + 1) * 512], kp[:])

    def vproj2(ch, dch, half):
        """V projection for two key blocks (one [128,512] psum tile, one
        strided copy into vv)."""
        vp = kvp.tile([128, 2, 256], f32, tag="kv", name="vp")
        for b in range(2):
            kb4 = half * 2 + b
            for c in range(2):
                nc.tensor.matmul(vp[:, b, :],
                                 dch[:, c, kb4 * 128:(kb4 + 1) * 128],
                                 wvT[:, c, :], start=(c == 0), stop=(c == 1),
                                 skip_group_check=True)
        eng = nc.scalar.copy if half == 0 else nc.vector.tensor_copy
        eng(vv_v[:, ch * 4 + half * 2:ch * 4 + half * 2 + 2, :, 0:64],
            vp[:].rearrange("p b (h e) -> p b h e", e=64))

    # pending half-done Schraudolph B-half: (e16 double tile, pt double
    # tile).  Two consecutive split tiles share one e16/pt pair so the
    # custom P2 correction runs once per PAIR ([128,1024]) on DVE.
    p2_pend = []

    def flush_p2():
        if not p2_pend:
            return
        e16d, pt2, nh = p2_pend.pop()
        nc.vector._custom_dve(exp_corr, out=pt2[:, 0:nh * LS],
                              in0=e16d[:, 0:nh * LS],
                              s0=mant_c, s1=U_COEF, imm2=V_COEF)

    def s_exp(kb, m):
        """S matmuls + exp for (kb, m).  The two head-halves go to two
        INDEPENDENT psum tiles: stA (h_even) is exp'd by ACT, stB (h_odd)
        takes the Schraudolph P1 on DVE + the custom P2 -- each half is
        freed by its single reader, so neither serializes the other.
        Returns (ptA, ptB) for the PV stage."""
        rt = route(kb, m)
        sta = stpa.tile([128, LS], f32, tag="sa", name="sta")
        nc.tensor.matmul(sta[:], ktT[:, m, kb * 128:(kb + 1) * 128],
                         qt[:, m, 0:LS], start=True, stop=True,
                         skip_group_check=True)
        stb = stpb.tile([128, LS], f32, tag="sb", name="stb")
        nc.tensor.matmul(stb[:], ktT[:, m, kb * 128:(kb + 1) * 128],
                         qt[:, m, LS:2 * LS], start=True, stop=True,
                         skip_group_check=True)
        pta = ptp.tile([128, LS], bf16, tag="pa", name="pta")
        nc.scalar.activation(pta[:], sta[:], Exp, scale=SCALE)
        if rt == "act":
            flush_p2()
            ptb = ptp.tile([128, LS], bf16, tag="pb", name="ptb")
            nc.scalar.activation(ptb[:], stb[:], Exp, scale=SCALE)
        else:
            if p2_pend:
                e16d, pt2, nh = p2_pend[0]
                nc.vector.tensor_scalar(e16d[:, LS:2 * LS].bitcast(i16),
                                        stb[:], A_P1, B_P1, MULT, ADD)
                p2_pend[0] = (e16d, pt2, 2)
                flush_p2()
                ptb = pt2[:, LS:2 * LS]
            else:
                e16d = e16p.tile([128, 2 * LS], bf16, tag="e16", name="e16")
                pt2 = ptp.tile([128, 2 * LS], bf16, tag="pb", name="pt2")
                nc.vector.tensor_scalar(e16d[:, 0:LS].bitcast(i16), stb[:],
                                        A_P1, B_P1, MULT, ADD)
                p2_pend.append((e16d, pt2, 1))
                ptb = pt2[:, 0:LS]
        return (pta, ptb)

    def emit_pv(job):
        kb, m, (pta, ptb) = job
        for j in range(2):
            h = 2 * m + j
            for qb in range(4):
                pv_matmul(h, qb, kb, pta if j == 0 else ptb, j)

    # ---- software-pipelined main loop ----
    # K/V projection for chunk ch+1 is interleaved between the attention
    # steps of chunk ch; PV for (kb, m) is emitted one tile-slot after its
    # S/exp so the PE never waits on a just-issued exp.
    from collections import deque

    pv_q = deque()
    PIPE = PIPE_N  # pending exp tiles before PV drains

    def drain(limit):
        while len(pv_q) > limit:
            emit_pv(pv_q.popleft())

    # PE p-state warmup (depends only on ones1) while the front DMAs land
    warm_t = stpa.tile([128, LS], f32, tag="sa", name="warm")
    for w in range(24):
        nc.tensor.matmul(warm_t[:, 0:128], ones1[0:1, :], ones1[0:1, :],
                         start=(w == 0), stop=(w == 23),
                         skip_group_check=True)

    # Q projection m=0 first, then chunk-0 K proj, so the kb0 S matmul's
    # inputs (qt m0 rows + ktT chunk 0) are ready as early as possible.
    st_q = [stpa.tile([128, LS], f32, tag="sa", name="st_q0"),
            stpb.tile([128, LS], f32, tag="sb", name="st_q1")]
    for k in range(4):
        nc.tensor.matmul(st_q[0][:], wqT[:, k, 0:128], latT[:, k, :],
                         start=(k == 0), stop=(k == 3),
                         skip_group_check=True)
    kproj(0, drs[0], 0)
    nc.scalar.copy(qt[0:64, 0, 0:LS], st_q[0][0:64, :])
    nc.vector.tensor_copy(qt[64:128, 0, LS:2 * LS], st_q[0][64:128, :])
    for k in range(4):
        nc.tensor.matmul(st_q[1][:], wqT[:, k, 128:256], latT[:, k, :],
                         start=(k == 0), stop=(k == 3),
                         skip_group_check=True)
    vproj2(0, drs[0], 0)
    kproj(0, drs[0], 1)
    nc.scalar.copy(qt[0:64, 1, 0:LS], st_q[1][0:64, :])
    nc.vector.tensor_copy(qt[64:128, 1, LS:2 * LS], st_q[1][64:128, :])
    vproj2(0, drs[0], 1)

    for ch in range(NCH):
        dch = drs.pop(ch)
        if ch + 3 < NCH:
            drs[ch + 3] = load_chunk(ch + 3)
        nxt = drs.get(ch + 1)
        # 4 next-chunk projection groups, one per kb (2.1us apart, so each
        # kvp-bank copy has plenty of time before the next group needs it)
        kv_jobs = deque()
        if nxt is not None:
            kv_jobs.extend([
                lambda m=0: kproj(ch + 1, nxt, m),
                lambda: vproj2(ch + 1, nxt, 0),
                lambda m=1: kproj(ch + 1, nxt, m),
                lambda: vproj2(ch + 1, nxt, 1),
            ])
        morder = (1, 0) if ch == NCH - 1 else (0, 1)
        for i in range(4):
            kb = ch * 4 + i
            for m in morder:
                pv_q.append((kb, m, s_exp(kb, m)))
                drain(PIPE)
                if kv_jobs and m == morder[1]:
                    kv_jobs.popleft()()
        if ch == 0:
            late_dmas()
    flush_p2()
    drain(0)

    # close S/KV psum pools; acc stays alive for the normalize reads
    for c in reversed(att_ctxs):
        c.__exit__(None, None, None)

    # ---- denominators -> reciprocals ----
    for t, n0, r0 in ((accA, 7, 0), (accB, 7, 7), (accC, 2, 14)):
        tv = t.rearrange("p (n e) -> p n e", e=65)
        nc.vector.reciprocal(rden[:, r0:r0 + n0], tv[:, :, 64])

    # ---- normalize + transpose + O proj, pipelined per q-block ----
    # Within each q-block the c=0 half (heads 0-1, fed by the earlier m=0
    # accumulators) runs first; the c=1 half rides the short critical path
    # from the very last PV.  The bias matmul opens each accumulation so it
    # is never on the critical path.
    with tc.tile_pool(name="fps", bufs=2, space="PSUM") as fps, \
         tc.tile_pool(name="tps", bufs=2, space="PSUM") as tps, \
         tc.tile_pool(name="obuf", bufs=4) as obuf:
        for qb in range(4):
            fp = fps.tile([128, OUT_DIM], f32, tag="fp", name="fp")
            nc.tensor.matmul(fp[:], ones1[0:1, :], bo_r[0:1, :],
                             start=True, stop=False, skip_group_check=True)
            tp = tps.tile([128, 2, 128], f32, tag="tp", name="tp")
            for c in range(2):
                for j in range(2):
                    h = 2 * c + j
                    r = h * 4 + qb
                    t, co = acc_region(r)
                    if j == 1:
                        nc.vector.tensor_scalar(
                            an[:, qb, h * 64:(h + 1) * 64],
                            t[:, co:co + 64], rden[:, r:r + 1], None, MULT)
                    else:
                        nc.scalar.mul(an[:, qb, h * 64:(h + 1) * 64],
                                      t[:, co:co + 64], rden[:, r:r + 1])
                nc.tensor.matmul(tp[:, c, :], an[:, qb, c * 128:(c + 1) * 128],
                                 ident[:], start=True, stop=True,
                                 skip_group_check=True)
                eng = nc.vector.tensor_copy if qb % 2 else nc.scalar.copy
                eng(anT[:, c, qb * 128:(qb + 1) * 128], tp[:, c, :])
                nc.tensor.matmul(fp[:], anT[:, c, qb * 128:(qb + 1) * 128],
                                 woT[:, c, :], start=False, stop=(c == 1),
                                 skip_group_check=True)
            ob = obuf.tile([128, OUT_DIM], bf16, tag="ob", name="ob")
            nc.scalar.copy(ob[:, 0:256], fp[:, 0:256])
            nc.vector.tensor_copy(ob[:, 256:512], fp[:, 256:512])
            nc.sync.dma_start(outp[:, qb, :], ob[:])

    accp_ctx.__exit__(None, None, None)


def build():
    if "nc" in _CACHE:
        return _CACHE["nc"]
    from contextlib import ExitStack

    import concourse.tile as tile
    from concourse import bacc

    nc = bacc.Bacc("TRN2", target_bir_lowering=False, debug=False,
                   num_devices=NCORES)
    with tile.TileContext(nc) as tc:
        with ExitStack() as ctx:
            _emit(ctx, tc, nc)
    nc.compile()
    _CACHE["nc"] = nc
    return nc


def _pm(a, nblk):
    """[nblk*128, f] -> partition-major [128, nblk, f]."""
    f = a.shape[1]
    return np.ascontiguousarray(
        a.reshape(nblk, 128, f).transpose(1, 0, 2))


def shard(inputs):
    import ml_dtypes

    data = np.asarray(inputs["data"], dtype=np.float32)
    latent = np.asarray(inputs["latent"], dtype=np.float32)
    wq = np.asarray(inputs["Wq"], dtype=np.float32)
    wk = np.asarray(inputs["Wk"], dtype=np.float32)
    wv = np.asarray(inputs["Wv"], dtype=np.float32)
    wo = np.asarray(inputs["Wo"], dtype=np.float32)
    bo = np.asarray(inputs["bo"], dtype=np.float32).reshape(1, OUT_DIM)

    bf = ml_dtypes.bfloat16
    dataTs = [_pm(data[b].T, 2).astype(bf) for b in range(B)]
    latTs = [_pm(np.ascontiguousarray(latent[b].T), 4).astype(bf)
             for b in range(B)]
    halves = []
    for g in range(2):
        hs = slice(g * 256, (g + 1) * 256)
        wqT = _pm(np.ascontiguousarray(wq[hs, :].T), 4).astype(bf)
        wkT = _pm(np.ascontiguousarray(wk[hs, :].T), 2).astype(bf)
        wvT = _pm(np.ascontiguousarray(wv[hs, :].T), 2).astype(bf)
        woT = _pm(np.ascontiguousarray(wo[:, hs].T), 2).astype(bf)
        boz = (bo if g == 0 else np.zeros_like(bo)).astype(bf)
        halves.append((wqT, wkT, wvT, woT, boz))

    in_maps = []
    for i in range(NCORES):
        b, g = i // 2, i % 2
        wqT, wkT, wvT, woT, boz = halves[g]
        in_maps.append({
            "dataT": dataTs[b], "latentT": latTs[b], "wqT": wqT,
            "wkT": wkT, "wvT": wvT, "woT": woT, "boz": boz,
        })
    return in_maps


def unshard(results):
    out = np.empty((B, LS, OUT_DIM), dtype=np.float32)
    for b in range(B):
        acc = None
        for g in range(2):
            o = results[2 * b + g]["outp"].astype(np.float32)
            o = o.transpose(1, 0, 2).reshape(LS, OUT_DIM)
            acc = o if acc is None else acc + o
        out[b] = acc
    return out


def run(inputs, trace=False):
    from concourse import bass_utils

    nc = build()
    in_maps = shard(inputs)
    res = bass_utils.run_bass_kernel_spmd(
        nc, in_maps, core_ids=list(range(NCORES)), trace=trace)
    return unshard(res.results), res


def kernel(**inputs):
    return run(inputs)[0]


# revision 7
# speedup vs baseline: 1.0308x; 1.0027x over previous
"""Trainium2 Bass kernel for the latent-query attention module (v2).

Math (fp32 inputs):
  Q = latent @ Wq.T; K = data @ Wk.T; V = data @ Wv.T
  S = (Q K^T)/sqrt(D); P = softmax_keys(S); out = (P V) @ Wo.T + bo

Sharding: 8 cores = 4 batches x 2 head-halves (4 heads each). Each core
computes K/V projections only for its 4 heads (zero duplicated work) and
a partial output  A_norm @ Wo[:, half].T (+ bo on even cores only, via a
zeros bias input on odd cores).  Host unshard = sum of the pair partials.

Per-core pipeline (streamed over 8 data chunks of 512 keys):
  PE   : K/V/Q projections; S via zero-packed head pairs (the two heads
         of a pair occupy disjoint 64-row/512-col quadrants of qt, so a
         single 128-deep matmul per half computes one head's S block);
         PV with P^T blocks STATIONARY and V moving (N=65 per matmul --
         half the PE cycles of the V-stationary orientation; the 65th V
         column of ones accumulates the softmax denominators); A^T via
         identity-matmul transposes; O projection.  All bf16.
  ACT  : exp for the h_even S half of every tile + some h_odd halves,
         kp/vv copy share, tail copies.
  DVE  : 2-pass exp for most h_odd halves: int16 Schraudolph P1
         (tensor_scalar writing the bf16 bit pattern of ~exp) + the
         custom 1-instruction P2 EXP_CORR_ANT (bitwise mantissa extract
         + linear correction, batched over tile pairs); normalize.
  Pool : SBUF-only work (memsets, iota) -- GPSIMD cannot touch PSUM.
The two S halves go to independent single-bank PSUM tiles so each is
freed by its one reader; PV emission trails S/exp by PIPE tiles.
All DRAM I/O is bf16 (except nothing -- data/latent/weights converted on
host), so no on-device dtype-conversion passes and half the DMA bytes.
"""

import sys

sys.path.insert(0, "/opt/trn_rl_repo")

import numpy as np

B, DS, DC = 4, 4096, 256
LS, LC = 512, 512
H, D = 8, 64
INNER, OUT_DIM = 512, 512
NCORES = 8
HL = 4                 # local heads per core
KB = DS // 128         # 32 key blocks
NCH = DS // 512        # 8 data chunks
SCALE = D ** -0.5

# Schraudolph (int16 / bf16-layout): i16 = trunc(A*s + B), bits = bf16 of
# ~exp(s/8); P2 corrects the (1+f) vs 2^f mantissa error.
A_P1 = 128.0 * np.log2(np.e) * SCALE
B_P1 = 127.0 * 128.0 + 0.5
MANT_MASK = 0x007F0000
U_COEF = -0.2429394
V_COEF = 0.2478516

_CACHE = {}

ROUTES = {0: "act", 1: "split_dve", 2: "split_dve", 3: "split_dve",
          4: "act", 5: "split_dve", 6: "act", 7: "split_dve"}
SPLIT_COL = 640
PIPE_N = 3


# ---------------------------------------------------------------------------
# custom DVE op: out = in0 * (1 + f*(u + v*f)),  f = mantissa(in0) in [0,1)
# ---------------------------------------------------------------------------
def _register_exp_corr():
    import concourse.dve_ops as dve_ops
    from concourse.dve_spec import AluOp, Bin, C0, C1, C2, One, Spec, lower
    from concourse.dve_uop import DveOpSpec

    if "EXP_CORR_ANT" in dve_ops._SUB_OPCODE_FOR_NAME:
        return next(op for op in dve_ops.OPS if op.name == "EXP_CORR_ANT")

    from concourse.dve_spec import Src0
    _g = Bin(AluOp.BITWISE_OR, Bin(AluOp.BITWISE_AND, Src0, C0), One)
    _x = _g - One
    _body = Src0 * (_x * (_x * C2 + C1) + One)

    def _ref(in0, in1, c0, c1, c2):
        b = in0.astype(np.float32).view(np.int32)
        c0b = np.float32(c0).view(np.int32)
        g = ((b & c0b) | np.float32(1.0).view(np.int32)).view(np.float32)
        xx = g - np.float32(1.0)
        k = np.float32(1.0) + xx * (np.float32(c1) + np.float32(c2) * xx)
        return (in0.astype(np.float32) * k).astype(np.float32)

    spec = Spec(body=_body, reference=_ref)
    shas = {}
    for ver in ("v3", "v4"):
        s = DveOpSpec(name="EXP_CORR_ANT", opcode=1, uops=lower(spec, ver=ver),
                      rd1_en=False)
        shas[ver] = s.sha(ver)
    op = dve_ops.DveOp.__new__(dve_ops.DveOp)
    object.__setattr__(op, "name", "EXP_CORR_ANT")
    object.__setattr__(op, "spec", spec)
    object.__setattr__(op, "subdim", False)
    object.__setattr__(op, "uops_sha", shas)
    object.__setattr__(op, "perf_en", {})
    dve_ops.OPS.append(op)
    dve_ops._SUB_OPCODE_FOR_NAME["EXP_CORR_ANT"] = (
        dve_ops._CUSTOM_DVE_ROW_BASE + len(dve_ops.OPS) - 1)
    dve_ops.CUSTOM_DVE_SPECS["EXP_CORR_ANT"] = spec
    return op


# ---------------------------------------------------------------------------
def _emit(ctx, tc, nc):
    from concourse import mybir
    from concourse.tile_rust import add_dep_helper

    f32 = mybir.dt.float32
    f32r = mybir.dt.float32r
    bf16 = mybir.dt.bfloat16
    i16 = mybir.dt.int16
    i32 = mybir.dt.int32
    Exp = mybir.ActivationFunctionType.Exp
    Identity = mybir.ActivationFunctionType.Identity
    MULT = mybir.AluOpType.mult
    ADD = mybir.AluOpType.add
    ISEQ = mybir.AluOpType.is_equal

    exp_corr = _register_exp_corr()
    mant_c = float(np.int32(MANT_MASK).view(np.float32))

    # ---- DRAM I/O ----
    dataT = nc.dram_tensor("dataT", [128, 2, DS], bf16, kind="ExternalInput").ap()
    latT_d = nc.dram_tensor("latentT", [128, 4, LS], bf16, kind="ExternalInput").ap()
    wqT_d = nc.dram_tensor("wqT", [128, 4, 256], bf16, kind="ExternalInput").ap()
    wkT_d = nc.dram_tensor("wkT", [128, 2, 256], bf16, kind="ExternalInput").ap()
    wvT_d = nc.dram_tensor("wvT", [128, 2, 256], bf16, kind="ExternalInput").ap()
    woT_d = nc.dram_tensor("woT", [128, 2, OUT_DIM], bf16, kind="ExternalInput").ap()
    boz_d = nc.dram_tensor("boz", [1, OUT_DIM], bf16, kind="ExternalInput").ap()
    outp = nc.dram_tensor("outp", [128, 4, OUT_DIM], bf16, kind="ExternalOutput").ap()

    # ---- resident SBUF ----
    res = ctx.enter_context(tc.tile_pool(name="res", bufs=1))
    ktT = res.tile([128, 2, DS], bf16, name="ktT")        # K^T, head pairs
    vv = res.tile([128, KB, HL * 65], bf16, name="vv")    # V + ones col
    vv_v = vv.rearrange("p k (h e) -> p k h e", e=65)
    qt = res.tile([128, 2, 2 * LS], bf16, name="qt")      # zero-packed Q^T
    ident = res.tile([128, 128], bf16, name="ident")
    ones1 = res.tile([1, 128], bf16, name="ones1")
    bo_r = res.tile([1, OUT_DIM], bf16, name="bo_r")
    bias_t = res.tile([128, 1], f32, name="bias_t")
    rden = res.tile([128, 16], f32, name="rden")
    an = res.tile([128, 4, 256], bf16, name="an")         # [q, qb, 4h*64]
    anT = res.tile([128, 2, LS], bf16, name="anT")        # [inner, c, q]

    wks = ctx.enter_context(tc.tile_pool(name="wks", bufs=1))
    latT = wks.tile([128, 4, LS], bf16, name="latT")
    wqT = wks.tile([128, 4, 256], bf16, name="wqT")
    wkT = wks.tile([128, 2, 256], bf16, name="wkT")
    wvT = wks.tile([128, 2, 256], bf16, name="wvT")
    woT = wks.tile([128, 2, OUT_DIM], bf16, name="woT")

    dstage = ctx.enter_context(tc.tile_pool(name="dstage", bufs=4))
    e16p = ctx.enter_context(tc.tile_pool(name="e16p", bufs=5))
    ptp = ctx.enter_context(tc.tile_pool(name="ptp", bufs=7))

    # ---- initial DMAs, split across 3 HWDGE queues so the front is
    # parallel: SP carries data chunks, ACT the weights, DVE the latent
    # (in lc-chunks so Q proj can start after the first chunk).
    drs = {}

    def load_chunk(ch):
        d = dstage.tile([128, 2, 512], bf16, tag="ds", name="dch")
        nc.sync.dma_start(d[:], dataT[:, :, ch * 512:(ch + 1) * 512])
        return d

    # SP queue: data chunk 0, latent chunks, then the data chunk stream.
    # ACT queue: all the weights.  Keeps the front parallel.
    nc.scalar.dma_start(wqT[:], wqT_d)
    nc.sync.dma_start(latT[:, 0:2, :], latT_d[:, 0:2, :])
    nc.sync.dma_start(latT[:, 2:4, :], latT_d[:, 2:4, :])
    nc.scalar.dma_start(wkT[:], wkT_d)
    drs[0] = load_chunk(0)
    nc.scalar.dma_start(wvT[:], wvT_d)
    drs[1] = load_chunk(1)
    drs[2] = load_chunk(2)

    def late_dmas():
        # tail-only tensors: emit mid-loop so their descriptor-gen slots
        # don't crowd the front HWDGE queue
        nc.scalar.dma_start(woT[:], woT_d)
        nc.scalar.dma_start(bo_r[:], boz_d)

    # ---- constants (ones first: the PE warmup depends only on it) ----
    nc.vector.memset(ones1[:], 1.0)
    consts = ctx.enter_context(tc.tile_pool(name="consts", bufs=1))
    iden_i = consts.tile([128, 128], i32, name="iden_i")
    nc.gpsimd.iota(iden_i[:], [[1, 128]], base=0, channel_multiplier=-1)
    nc.vector.tensor_scalar(ident[:], iden_i[:], 0, None, ISEQ)
    nc.vector.memset(bias_t[:], B_P1)
    nc.vector.memset(qt[:], 0.0)
    nc.gpsimd.memset(vv_v[:, :, :, 64:65], 1.0)

    # ---- attention-scope PSUM pools (8 banks total) ----
    # stp: 2 tiles x [128,1024] f32 = 4 banks; acc: 3 banks; kvp: 1 bank
    accp_ctx = tc.tile_pool(name="accp", bufs=1, space="PSUM")
    accp = accp_ctx.__enter__()
    att_ctxs = [tc.tile_pool(name="stpa", bufs=2, space="PSUM"),
                tc.tile_pool(name="stpb", bufs=2, space="PSUM"),
                tc.tile_pool(name="kvp", bufs=1, space="PSUM")]
    stpa, stpb, kvp = [c.__enter__() for c in att_ctxs]
    accA = accp.tile([128, 455], f32, name="accA")
    accB = accp.tile([128, 455], f32, name="accB")
    accC = accp.tile([128, 130], f32, name="accC")
    acc_tiles = [accA, accB, accC]

    def acc_region(r):
        """region r (= h*4+qb) -> (tile, col offset). 7+7+2 packing."""
        t = r // 7 if r < 14 else 2
        c = (r % 7 if r < 14 else r - 14) * 65
        return acc_tiles[t], c

    first_bank_mm = {}

    def pv_matmul(h, qb, kb, pt, j):
        r = h * 4 + qb
        t, c = acc_region(r)
        first = (kb == 0)
        is_clearing = first and (r % 7 == 0 or r == 14)
        mm = nc.tensor.matmul(
            t[:, c:c + 65],
            pt[:, qb * 128:(qb + 1) * 128],
            vv[:, kb, h * 65:h * 65 + 65],
            start=is_clearing, stop=(kb == KB - 1),
            skip_group_check=True)
        ti = t.name if hasattr(t, "name") else id(t)
        if is_clearing:
            first_bank_mm[ti] = mm
        elif first and ti in first_bank_mm:
            add_dep_helper(mm.ins, first_bank_mm[ti].ins, sync=False,
                           reason="acc bank-clear order")
        return mm

    # exp route per (kb, m): "act" = whole tile on ACT; "split_dve"/
    # "split_pool" = ACT exps cols 0:SPL while Schraudolph P1 runs on cols
    # SPL:1024 on the named engine IN PARALLEL (with only 2 st PSUM slots
    # the st-occupancy of the exp stage is the pipeline's critical chain;
    # the parallel split frees st after ~0.8us instead of 1.0-1.5us), then
    # the custom DVE P2 finishes the Schraudolph part off the chain.
    SPL = SPLIT_COL

    def route(kb, m):
        i = (kb % 4) * 2 + m
        return ROUTES[i]

    def kproj(ch, dch, m):
        kp = kvp.tile([128, 512], f32, tag="kv", name="kp")
        for c in range(2):
            nc.tensor.matmul(kp[:], wkT[:, c, m * 128:(m + 1) * 128],
                             dch[:, c, :], start=(c == 0), stop=(c == 1))
        eng = nc.vector.tensor_copy if m == 0 else nc.scalar.copy
        eng(ktT[:, m, ch * 512:(ch + 1) * 512], kp[:])

    def vproj2(ch, dch, half):
        """V projection for two key blocks (one [128,512] psum tile, one
        strided copy into vv)."""
        vp = kvp.tile([128, 2, 256], f32, tag="kv", name="vp")
        for b in range(2):
            kb4 = half * 2 + b
            for c in range(2):
                nc.tensor.matmul(vp[:, b, :],
                                 dch[:, c, kb4 * 128:(kb4 + 1) * 128],
                                 wvT[:, c, :], start=(c == 0), stop=(c == 1),
                                 skip_group_check=True)
        eng = nc.scalar.copy if half == 0 else nc.vector.tensor_copy
        eng(vv_v[:, ch * 4 + half * 2:ch * 4 + half * 2 + 2, :, 0:64],
            vp[:].rearrange("p b (h e) -> p b h e", e=64))

    # pending half-done Schraudolph B-half: (e16 double tile, pt double
    # tile).  Two consecutive split tiles share one e16/pt pair so the
    # custom P2 correction runs once per PAIR ([128,1024]) on DVE.
    p2_pend = []

    def flush_p2():
        if not p2_pend:
            return
        e16d, pt2, nh = p2_pend.pop()
        nc.vector._custom_dve(exp_corr, out=pt2[:, 0:nh * LS],
                              in0=e16d[:, 0:nh * LS],
                              s0=mant_c, s1=U_COEF, imm2=V_COEF)

    def s_exp(kb, m):
        """S matmuls + exp for (kb, m).  The two head-halves go to two
        INDEPENDENT psum tiles: stA (h_even) is exp'd by ACT, stB (h_odd)
        takes the Schraudolph P1 on DVE + the custom P2 -- each half is
        freed by its single reader, so neither serializes the other.
        Returns (ptA, ptB) for the PV stage."""
        rt = route(kb, m)
        sta = stpa.tile([128, LS], f32, tag="sa", name="sta")
        nc.tensor.matmul(sta[:], ktT[:, m, kb * 128:(kb + 1) * 128],
                         qt[:, m, 0:LS], start=True, stop=True,
                         skip_group_check=True)
        stb = stpb.tile([128, LS], f32, tag="sb", name="stb")
        nc.tensor.matmul(stb[:], ktT[:, m, kb * 128:(kb + 1) * 128],
                         qt[:, m, LS:2 * LS], start=True, stop=True,
                         skip_group_check=True)
        pta = ptp.tile([128, LS], bf16, tag="pa", name="pta")
        nc.scalar.activation(pta[:], sta[:], Exp, scale=SCALE)
        if rt == "act":
            flush_p2()
            ptb = ptp.tile([128, LS], bf16, tag="pb", name="ptb")
            nc.scalar.activation(ptb[:], stb[:], Exp, scale=SCALE)
        else:
            if p2_pend:
                e16d, pt2, nh = p2_pend[0]
                nc.vector.tensor_scalar(e16d[:, LS:2 * LS].bitcast(i16),
                                        stb[:], A_P1, B_P1, MULT, ADD)
                p2_pend[0] = (e16d, pt2, 2)
                flush_p2()
                ptb = pt2[:, LS:2 * LS]
            else:
                e16d = e16p.tile([128, 2 * LS], bf16, tag="e16", name="e16")
                pt2 = ptp.tile([128, 2 * LS], bf16, tag="pb", name="pt2")
                nc.vector.tensor_scalar(e16d[:, 0:LS].bitcast(i16), stb[:],
                                        A_P1, B_P1, MULT, ADD)
                p2_pend.append((e16d, pt2, 1))
                ptb = pt2[:, 0:LS]
        return (pta, ptb)

    def emit_pv(job):
        kb, m, (pta, ptb) = job
        for j in range(2):
            h = 2 * m + j
            for qb in range(4):
                pv_matmul(h, qb, kb, pta if j == 0 else ptb, j)

    # ---- software-pipelined main loop ----
    # K/V projection for chunk ch+1 is interleaved between the attention
    # steps of chunk ch; PV for (kb, m) is emitted one tile-slot after its
    # S/exp so the PE never waits on a just-issued exp.
    from collections import deque

    pv_q = deque()
    PIPE = PIPE_N  # pending exp tiles before PV drains

    def drain(limit):
        while len(pv_q) > limit:
            emit_pv(pv_q.popleft())

    # PE p-state warmup (depends only on ones1) while the front DMAs land
    warm_t = stpa.tile([128, LS], f32, tag="sa", name="warm")
    for w in range(24):
        nc.tensor.matmul(warm_t[:, 0:128], ones1[0:1, :], ones1[0:1, :],
                         start=(w == 0), stop=(w == 23),
                         skip_group_check=True)

    # Q projection m=0 first, then chunk-0 K proj, so the kb0 S matmul's
    # inputs (qt m0 rows + ktT chunk 0) are ready as early as possible.
    st_q = [stpa.tile([128, LS], f32, tag="sa", name="st_q0"),
            stpb.tile([128, LS], f32, tag="sb", name="st_q1")]
    for k in range(4):
        nc.tensor.matmul(st_q[0][:], wqT[:, k, 0:128], latT[:, k, :],
                         start=(k == 0), stop=(k == 3),
                         skip_group_check=True)
    kproj(0, drs[0], 0)
    nc.scalar.copy(qt[0:64, 0, 0:LS], st_q[0][0:64, :])
    nc.vector.tensor_copy(qt[64:128, 0, LS:2 * LS], st_q[0][64:128, :])
    for k in range(4):
        nc.tensor.matmul(st_q[1][:], wqT[:, k, 128:256], latT[:, k, :],
                         start=(k == 0), stop=(k == 3),
                         skip_group_check=True)
    vproj2(0, drs[0], 0)
    kproj(0, drs[0], 1)
    nc.scalar.copy(qt[0:64, 1, 0:LS], st_q[1][0:64, :])
    nc.vector.tensor_copy(qt[64:128, 1, LS:2 * LS], st_q[1][64:128, :])
    vproj2(0, drs[0], 1)

    for ch in range(NCH):
        dch = drs.pop(ch)
        if ch + 3 < NCH:
            drs[ch + 3] = load_chunk(ch + 3)
        nxt = drs.get(ch + 1)
        # 4 next-chunk projection groups, one per kb (2.1us apart, so each
        # kvp-bank copy has plenty of time before the next group needs it)
        kv_jobs = deque()
        if nxt is not None:
            kv_jobs.extend([
                lambda m=0: kproj(ch + 1, nxt, m),
                lambda: vproj2(ch + 1, nxt, 0),
                lambda m=1: kproj(ch + 1, nxt, m),
                lambda: vproj2(ch + 1, nxt, 1),
            ])
        morder = (1, 0) if ch == NCH - 1 else (0, 1)
        for i in range(4):
            kb = ch * 4 + i
            for m in morder:
                pv_q.append((kb, m, s_exp(kb, m)))
                drain(PIPE)
                if kv_jobs and m == morder[1]:
                    kv_jobs.popleft()()
        if ch == 0:
            late_dmas()
    flush_p2()
    drain(0)

    # close S/KV psum pools; acc stays alive for the normalize reads
    for c in reversed(att_ctxs):
        c.__exit__(None, None, None)

    # ---- denominators -> reciprocals ----
    for t, n0, r0 in ((accA, 7, 0), (accB, 7, 7), (accC, 2, 14)):
        tv = t.rearrange("p (n e) -> p n e", e=65)
        nc.vector.reciprocal(rden[:, r0:r0 + n0], tv[:, :, 64])

    # ---- normalize + transpose + O proj, pipelined per q-block ----
    # Within each q-block the c=0 half (heads 0-1, fed by the earlier m=0
    # accumulators) runs first; the c=1 half rides the short critical path
    # from the very last PV.  The bias matmul opens each accumulation so it
    # is never on the critical path.
    with tc.tile_pool(name="fps", bufs=2, space="PSUM") as fps, \
         tc.tile_pool(name="tps", bufs=2, space="PSUM") as tps, \
         tc.tile_pool(name="obuf", bufs=4) as obuf:
        for qb in range(4):
            fp = fps.tile([128, OUT_DIM], f32, tag="fp", name="fp")
            nc.tensor.matmul(fp[:], ones1[0:1, :], bo_r[0:1, :],
                             start=True, stop=False, skip_group_check=True)
            tp = tps.tile([128, 2, 128], f32, tag="tp", name="tp")
            for c in range(2):
                for j in range(2):
                    h = 2 * c + j
                    r = h * 4 + qb
                    t, co = acc_region(r)
                    if j == 1:
                        nc.vector.tensor_scalar(
                            an[:, qb, h * 64:(h + 1) * 64],
                            t[:, co:co + 64], rden[:, r:r + 1], None, MULT)
                    else:
                        nc.scalar.mul(an[:, qb, h * 64:(h + 1) * 64],
                                      t[:, co:co + 64], rden[:, r:r + 1])
                nc.tensor.matmul(tp[:, c, :], an[:, qb, c * 128:(c + 1) * 128],
                                 ident[:], start=True, stop=True,
                                 skip_group_check=True)
                eng = nc.vector.tensor_copy if qb % 2 else nc.scalar.copy
                eng(anT[:, c, qb * 128:(qb + 1) * 128], tp[:, c, :])
                nc.tensor.matmul(fp[:], anT[:, c, qb * 128:(qb + 1) * 128],
                                 woT[:, c, :], start=False, stop=(c == 1),
                                 skip_group_check=True)
            ob = obuf.tile([128, OUT_DIM], bf16, tag="ob", name="ob")
            nc.scalar.copy(ob[:, 0:256], fp[:, 0:256])
            nc.vector.tensor_copy(ob[:, 256:512], fp[:, 256:512])
            nc.sync.dma_start(outp[:, qb, :], ob[:])

    accp_ctx.__exit__(None, None, None)


def build():
    if "nc" in _CACHE:
        return _CACHE["nc"]
    from contextlib import ExitStack

    import concourse.tile as tile
    from concourse import bacc

    nc = bacc.Bacc("TRN2", target_bir_lowering=False, debug=False,
                   num_devices=NCORES)
    with tile.TileContext(nc) as tc:
        with ExitStack() as ctx:
            _emit(ctx, tc, nc)
    nc.compile()
    _CACHE["nc"] = nc
    return nc


def _pm(a, nblk):
    """[nblk*128, f] -> partition-major [128, nblk, f]."""
    f = a.shape[1]
    return np.ascontiguousarray(
        a.reshape(nblk, 128, f).transpose(1, 0, 2))


def shard(inputs):
    import ml_dtypes

    data = np.asarray(inputs["data"], dtype=np.float32)
    latent = np.asarray(inputs["latent"], dtype=np.float32)
    wq = np.asarray(inputs["Wq"], dtype=np.float32)
    wk = np.asarray(inputs["Wk"], dtype=np.float32)
    wv = np.asarray(inputs["Wv"], dtype=np.float32)
    wo = np.asarray(inputs["Wo"], dtype=np.float32)
    bo = np.asarray(inputs["bo"], dtype=np.float32).reshape(1, OUT_DIM)

    bf = ml_dtypes.bfloat16
    dataTs = [_pm(data[b].T, 2).astype(bf) for b in range(B)]
    latTs = [_pm(np.ascontiguousarray(latent[b].T), 4).astype(bf)
             for b in range(B)]
    halves = []
    for g in range(2):
        hs = slice(g * 256, (g + 1) * 256)
        wqT = _pm(np.ascontiguousarray(wq[hs, :].T), 4).astype(bf)
        wkT = _pm(np.ascontiguousarray(wk[hs, :].T), 2).astype(bf)
        wvT = _pm(np.ascontiguousarray(wv[hs, :].T), 2).astype(bf)
        woT = _pm(np.ascontiguousarray(wo[:, hs].T), 2).astype(bf)
        boz = (bo if g == 0 else np.zeros_like(bo)).astype(bf)
        halves.append((wqT, wkT, wvT, woT, boz))

    in_maps = []
    for i in range(NCORES):
        b, g = i // 2, i % 2
        wqT, wkT, wvT, woT, boz = halves[g]
        in_maps.append({
            "dataT": dataTs[b], "latentT": latTs[b], "wqT": wqT,
            "wkT": wkT, "wvT": wvT, "woT": woT, "boz": boz,
        })
    return in_maps


def unshard(results):
    out = np.empty((B, LS, OUT_DIM), dtype=np.float32)
    for b in range(B):
        acc = None
        for g in range(2):
            o = results[2 * b + g]["outp"].astype(np.float32)
            o = o.transpose(1, 0, 2).reshape(LS, OUT_DIM)
            acc = o if acc is None else acc + o
        out[b] = acc
    return out


def run(inputs, trace=False):
    from concourse import bass_utils

    nc = build()
    in_maps = shard(inputs)
    res = bass_utils.run_bass_kernel_spmd(
        nc, in_maps, core_ids=list(range(NCORES)), trace=trace)
    return unshard(res.results), res


def kernel(**inputs):
    return run(inputs)[0]
